# revision 1
# baseline (speedup 1.0000x reference)
"""Trainium2 Bass kernel for BERT-style CLS attention head.

Model (see harness reference):
  q/k/v projections of hidden [B=16, S=1024, H=768], 8 heads x 96,
  softmax attention, but ONLY the CLS token (query position 0) feeds the
  output projection  out = relu(ctx[:, 0] @ Wo + bo)  with Wo [768, 4].

Algebraic structure exploited on-device (per batch b):
  q~      = X[0] @ Wq + bq                      (only row 0 of Q needed)
  Qblk    [768, 16] = diag-blocked q~ / sqrt(96)  (head masks, host const)
  Z^T     [16, 768] = Qblk.T @ WkT              (K-projection collapses:
                                                 768x768x16 instead of
                                                 768x768x1024 per batch)
  scores  [8, 1024] = Z.T @ X^T + mask          (bk shifts every row by a
                                                 constant -> cancels in
                                                 softmax; mask applied via a
                                                 rank-1 accumulating matmul)
  probs   = softmax(scores)                     (exp on ACT, row sums via
                                                 accum_out)
  r       [8, 768]  = probs_unnorm.T @ X        (X used in natural layout;
                                                 V never materialized)
  out     [4]       = relu(sum_h r_h/rowsum_h @ G_h + bo_eff)
  where G_h = Wv[:, h] @ Wo[h, :] and bo_eff = bo + bv @ Wo are fused
  weight constants computed on host (weight-only preprocessing).

Sharding: data-parallel over batch, 2 batches per core on 8 cores.
All matmuls run as float32r (FP22 multiplies, fp32 accumulate).

DMA/PE orchestration: constants are packed into two buffers (one DMA
each) to avoid per-transfer fixed costs at the head of the queue; the
queue order is consts -> Wq -> X[b0 first half] -> WkT -> remaining X
halves, and the PE stream is ordered so X^T transposes and score matmuls
consume each X half as it lands.
"""

import numpy as np

from concourse import bacc
import concourse.mybir as mybir
import concourse.tile as tile
from concourse.bass import _add_dep_helper
from concourse.bass_utils import run_bass_kernel_spmd

F32 = mybir.dt.float32
F32R = mybir.dt.float32r

B, S, H = 16, 1024, 768
NH, DH, O = 8, 96, 4
NCORES = 8
BL = B // NCORES          # 2 batches per core
C6 = H // 128             # 6 hidden chunks of 128
K8 = S // 128             # 8 sequence chunks of 128

# rowvec packing (one partition-0 row): ones | bq | boeff | amask b0 | amask b1
RV_ONES = 0
RV_BQ = 128
RV_BOEFF = RV_BQ + H          # 896
RV_AM0 = RV_BOEFF + O         # 900
RV_AM1 = RV_AM0 + S           # 1924
RV_LEN = RV_AM1 + S           # 2948

# kwide packing [128, .]: ident | x0t | qmask | gsb
KW_IDENT = 0
KW_X0T = 128
KW_QMASK = KW_X0T + C6 * BL   # 140
KW_GSB = KW_QMASK + C6 * NH   # 188
KW_LEN = KW_GSB + NH * C6 * O  # 380


def _r(ap):
    return ap.bitcast(F32R)


def build_program():
    nc = bacc.Bacc(None)

    hid = nc.declare_dram_parameter("hid", [BL, S, H], F32, isOutput=False)
    wq = nc.declare_dram_parameter("wq", [H, H], F32, isOutput=False)
    wkt = nc.declare_dram_parameter("wkt", [H, H], F32, isOutput=False)
    kwide = nc.declare_dram_parameter("kwide", [128, KW_LEN], F32, isOutput=False)
    out_d = nc.declare_dram_parameter("out", [BL, O], F32, isOutput=True)

    with tile.TileContext(nc) as tc:
        with (
            tc.tile_pool(name="konst", bufs=1) as kp,
            tc.tile_pool(name="work", bufs=1) as wp,
            tc.tile_pool(name="tps", bufs=3, space="PSUM") as tpsp,
            tc.tile_pool(name="acc", bufs=2, space="PSUM") as accp,
            tc.tile_pool(name="jnk", bufs=1, space="PSUM") as jp,
        ):
            # ---- persistent SBUF tiles ----
            kw_sb = kp.tile([128, KW_LEN], F32)
            ident_r = kp.tile([128, 128], F32)
            wq_sb = kp.tile([128, C6, H], F32)
            wkt_sb = kp.tile([128, C6, H], F32)
            x_sb = kp.tile([128, BL, K8, H], F32)
            xt_sb = kp.tile([128, BL, C6, S], F32)

            ident_v = kw_sb[:, KW_IDENT : KW_IDENT + 128]
            x0t_v = kw_sb[:, KW_X0T : KW_QMASK].rearrange("p (c b) -> p c b", c=C6)
            qmask_v = kw_sb[:, KW_QMASK : KW_GSB].rearrange("p (c h) -> p c h", c=C6)
            g_v = kw_sb[:, KW_GSB : KW_LEN].rearrange("p (a o) -> p a o", o=O)

            # ---- DMA queue (one HWDGE ring; completes in order) ----
            d_kw = nc.sync.dma_start(out=_r(kw_sb[:, :]), in_=_r(kwide[:, :]))
            d_idr = nc.sync.dma_start(
                out=_r(ident_r[:, :]), in_=_r(kwide[:, KW_IDENT : KW_IDENT + 128])
            )
            d_wq = nc.sync.dma_start(
                out=_r(wq_sb[:, :, :]),
                in_=_r(wq.rearrange("(c p) n -> p c n", p=128)),
            )

            def load_x(b, kq):
                return nc.sync.dma_start(
                    out=_r(x_sb[:, b, 4 * kq : 4 * kq + 4, :]),
                    in_=_r(
                        hid[b, 512 * kq : 512 * (kq + 1), :].rearrange(
                            "(k p) i -> p k i", p=128
                        )
                    ),
                )

            d_x00 = load_x(0, 0)
            d_wkt = nc.sync.dma_start(
                out=_r(wkt_sb[:, :, :]),
                in_=_r(wkt.rearrange("(c p) n -> p c n", p=128)),
            )
            d_x01 = load_x(0, 1)
            d_x10 = load_x(1, 0)
            d_x11 = load_x(1, 1)
            # stagger the big transfers: each waits on the one TWO back,
            # keeping two in flight (full HBM bandwidth) while completions
            # land in priority order
            deps = [
                (d_x00, d_wq),  # two transfers in flight at all times,
                (d_wkt, d_wq),  # completing in consumption order
                (d_x01, d_x00),
                (d_x10, d_wkt),
                (d_x11, d_x01),
            ]
            for later, earlier in deps:
                _add_dep_helper(
                    later.ins, earlier.ins, sync=True, reason="dma priority order"
                )

            # ---- PE warmup: junk matmuls while waiting for Wq ----
            # (HAM unthrottles the PE clock 1.2->2.4 GHz after ~3.4us of
            # sustained matmul activity; burn the DMA wait to get there)
            warm_ps = jp.tile([128, 512], F32)
            for _ in range(24):
                nc.tensor.matmul(
                    warm_ps[:, :KW_LEN], _r(ident_r[:, :]), _r(kw_sb[:, :])
                )

            # ---- q~ = X[0,:] @ Wq + bq  for both batches: [BL, H] ----
            q_ps = accp.tile([BL, H], F32, tag="acc")
            for n0, nw in ((0, 512), (512, 256)):
                for c in range(C6):
                    nc.tensor.matmul(
                        q_ps[:, n0 : n0 + nw],
                        _r(x0t_v[:, c, :]),
                        _r(wq_sb[:, c, n0 : n0 + nw]),
                        start=(c == 0),
                        stop=(c == C6 - 1),
                    )
            q_sb = wp.tile([BL, H], F32)
            nc.vector.tensor_copy(q_sb[:, :], q_ps[:, :])

            # ---- qT via PE transposes, fused with Qblk = qT * headmask ----
            qblk = wp.tile([128, C6, BL, NH], F32)
            for c in range(C6):
                qt_ps = tpsp.tile([128, 512], F32, tag="tps", name=f"qt_ps{c}")
                nc.tensor.transpose(
                    qt_ps[:, :BL], q_sb[:, 128 * c : 128 * (c + 1)], ident_v[:BL, :BL]
                )
                nc.vector.tensor_mul(
                    _r(qblk[:, c, :, :]),
                    qt_ps[:, :BL].unsqueeze(2).to_broadcast([128, BL, NH]),
                    qmask_v[:, c, :].unsqueeze(1).to_broadcast([128, BL, NH]),
                )

            # helpers -------------------------------------------------
            def xt_block(b, nh2):
                """4 PE transposes + 1 copy per i-chunk for one X half."""
                for ic in range(C6):
                    xt_ps = tpsp.tile(
                        [128, 512], F32, tag="tps", name=f"xt_ps{b}_{ic}_{nh2}"
                    )
                    for t in range(4):
                        k = 4 * nh2 + t
                        nc.tensor.transpose(
                            _r(xt_ps[:, 128 * t : 128 * (t + 1)]),
                            _r(x_sb[:, b, k, 128 * ic : 128 * (ic + 1)]),
                            _r(ident_r[:, :]),
                        )
                    if ic % 3 == 2:
                        nc.scalar.copy(
                            _r(xt_sb[:, b, ic, 512 * nh2 : 512 * (nh2 + 1)]),
                            _r(xt_ps[:, :]),
                        )
                    else:
                        nc.vector.tensor_copy(
                            _r(xt_sb[:, b, ic, 512 * nh2 : 512 * (nh2 + 1)]),
                            _r(xt_ps[:, :]),
                        )
                    # HAM anchor: PE transposes don't register as matmul
                    # activity; one real matmul per chunk keeps the clock
                    # unthrottled through transpose-heavy stretches
                    nc.tensor.matmul(
                        warm_ps[:, :KW_LEN], _r(ident_r[:, :]), _r(kw_sb[:, :])
                    )

            def sc_bank(b, sc_ps, z_sb, nh2):
                """scores bank nh2 for batch b: accumulate over i-chunks."""
                for ic in range(C6):
                    nc.tensor.matmul(
                        sc_ps[:, 512 * nh2 : 512 * (nh2 + 1)],
                        _r(z_sb[:, ic, NH * b : NH * (b + 1)]),
                        _r(xt_sb[:, b, ic, 512 * nh2 : 512 * (nh2 + 1)]),
                        start=(ic == 0),
                        stop=(ic == C6 - 1),
                    )

            def softmax(b, sc_ps):
                # scores are O(5) for this model; exp without max-sub is
                # exact w.r.t. the reference softmax (shift-invariant)
                probs = wp.tile([NH, S], F32, name=f"probs{b}")
                rowsum = wp.tile([NH, 1], F32, name=f"rowsum{b}")
                nc.scalar.activation(
                    probs[:, :],
                    sc_ps[:, :],
                    mybir.ActivationFunctionType.Exp,
                    bias=0.0,
                    scale=1.0,
                    accum_out=rowsum[:, :],
                )
                recip = wp.tile([NH, 1], F32, name=f"recip{b}")
                nc.vector.reciprocal(recip[:, :], rowsum[:, :])
                return probs, recip

            def pt_block(b, probs, pt_sb):
                for k in range(K8):
                    pt_ps = tpsp.tile([128, 512], F32, tag="tps", name=f"pt_ps{b}_{k}")
                    nc.tensor.transpose(
                        pt_ps[:, :NH],
                        probs[:, 128 * k : 128 * (k + 1)],
                        ident_v[:NH, :NH],
                    )
                    nc.vector.tensor_copy(_r(pt_sb[:, b, k, :]), pt_ps[:, :NH])

            def r_block(b, pt_sb, recip):
                r_ps = accp.tile([NH, H], F32, tag="acc", name=f"r_ps{b}")
                for n0, nw in ((0, 512), (512, 256)):
                    for k in range(K8):
                        nc.tensor.matmul(
                            r_ps[:, n0 : n0 + nw],
                            _r(pt_sb[:, b, k, :]),
                            _r(x_sb[:, b, k, n0 : n0 + nw]),
                            start=(k == 0),
                            stop=(k == K8 - 1),
                        )
                r_sb = wp.tile([NH, H], F32, name=f"r_sb{b}")
                nc.vector.tensor_scalar_mul(r_sb[:, :], r_ps[:, :], recip[:, :])
                return r_sb

            def rt_block(b, r_sb, rt_sb):
                for c in range(C6):
                    rt_ps = tpsp.tile([128, 512], F32, tag="tps", name=f"rt_ps{b}_{c}")
                    nc.tensor.transpose(
                        rt_ps[:, :NH],
                        r_sb[:, 128 * c : 128 * (c + 1)],
                        ident_v[:NH, :NH],
                    )
                    nc.vector.tensor_copy(_r(rt_sb[:, c, :, b]), rt_ps[:, :NH])

            def final_mms(b):
                outsum = accp.tile([1, O], F32, tag="acc", name=f"outsum{b}")
                n_mm = NH * C6
                i = 0
                for h in range(NH):
                    for c in range(C6):
                        i += 1
                        nc.tensor.matmul(
                            outsum[:, :],
                            _r(rt_sb[:, c, h, b : b + 1]),
                            _r(g_v[:, h * C6 + c, :]),
                            start=(i == 1),
                            stop=(i == n_mm),
                        )
                out_sb = wp.tile([1, O], F32, name=f"out_sb{b}")
                nc.vector.tensor_scalar_max(out_sb[:, :], outsum[:, :], 0.0)
                nc.sync.dma_start(out=out_d[b : b + 1, :], in_=out_sb[:, :])

            # ---- PE stream, ordered to chase the DMA queue ----------
            rt_sb = wp.tile([128, C6, NH, BL], F32)
            pt_sb = wp.tile([128, BL, K8, NH], F32)

            # X^T for batch 0 first half (arrives right after Wq)
            xt_block(0, 0)

            # Z^T [16, 768] = Qblk.T @ WkT, then transpose to Z [768, 16]
            zt_ps = accp.tile([BL * NH, H], F32, tag="acc")
            for n0, nw in ((0, 512), (512, 256)):
                for jc in range(C6):
                    nc.tensor.matmul(
                        zt_ps[:, n0 : n0 + nw],
                        _r(qblk[:, jc, :, :]),
                        _r(wkt_sb[:, jc, n0 : n0 + nw]),
                        start=(jc == 0),
                        stop=(jc == C6 - 1),
                    )
            zt_sb = wp.tile([BL * NH, H], F32)
            nc.vector.tensor_copy(zt_sb[:, :], zt_ps[:, :])
            z_sb = wp.tile([128, C6, BL * NH], F32)
            for it in range(C6):
                z_tps = tpsp.tile([128, 512], F32, tag="tps", name=f"z_tps{it}")
                nc.tensor.transpose(
                    z_tps[:, : BL * NH],
                    zt_sb[:, 128 * it : 128 * (it + 1)],
                    ident_v[: BL * NH, : BL * NH],
                )
                nc.vector.tensor_copy(_r(z_sb[:, it, :]), z_tps[:, : BL * NH])

            # batch 0: scores bank 0, then second X half + bank 1
            sc_ps0 = accp.tile([NH, S], F32, tag="acc", name="sc_ps0")
            sc_bank(0, sc_ps0, z_sb, 0)
            xt_block(0, 1)
            sc_bank(0, sc_ps0, z_sb, 1)
            probs0, recip0 = softmax(0, sc_ps0)

            # batch 1 X^T + scores ASAP (DMA-gated); b0 tail fills gaps
            xt_block(1, 0)
            sc_ps1 = accp.tile([NH, S], F32, tag="acc", name="sc_ps1")
            sc_bank(1, sc_ps1, z_sb, 0)
            xt_block(1, 1)
            sc_bank(1, sc_ps1, z_sb, 1)
            probs1, recip1 = softmax(1, sc_ps1)

            pt_block(0, probs0, pt_sb)
            r_sb0 = r_block(0, pt_sb, recip0)
            rt_block(0, r_sb0, rt_sb)
            final_mms(0)
            pt_block(1, probs1, pt_sb)
            r_sb1 = r_block(1, pt_sb, recip1)
            rt_block(1, r_sb1, rt_sb)
            final_mms(1)

    nc.finalize()
    return nc


_NC_CACHE = None


def _get_program():
    global _NC_CACHE
    if _NC_CACHE is None:
        _NC_CACHE = build_program()
    return _NC_CACHE


def _host_prep(inputs):
    """Weight fusion + layout prep (host side, weight/layout-only)."""
    hs = np.ascontiguousarray(np.asarray(inputs["hidden_states"], np.float32))
    am = np.ascontiguousarray(np.asarray(inputs["attention_mask"], np.float32))
    Wq = np.ascontiguousarray(np.asarray(inputs["Wq"], np.float32))
    bq = np.asarray(inputs["bq"], np.float32)
    Wk = np.asarray(inputs["Wk"], np.float32)
    Wv = np.asarray(inputs["Wv"], np.float32)
    bv = np.asarray(inputs["bv"], np.float32)
    Wo = np.asarray(inputs["Wo"], np.float32)
    bo = np.asarray(inputs["bo"], np.float32)

    wkt = np.ascontiguousarray(Wk.T)

    # G_h = Wv[:, h] @ Wo[h, :]; gsb[p, (h*C6+c)*O + o] = G_h[128c+p, o]
    g_sb = np.empty((128, NH * C6, O), np.float32)
    for h in range(NH):
        Gh = Wv[:, DH * h : DH * (h + 1)] @ Wo[DH * h : DH * (h + 1), :]
        g_sb[:, h * C6 : (h + 1) * C6, :] = Gh.reshape(C6, 128, O).transpose(1, 0, 2)

    boeff = (bo + bv @ Wo).astype(np.float32)

    # head mask with 1/sqrt(DH) folded in: [p, c*NH + h]
    j = np.arange(H)
    qmask = np.zeros((H, NH), np.float32)
    qmask[j, j // DH] = 1.0 / np.sqrt(np.float32(DH))
    qmask = qmask.reshape(C6, 128, NH).transpose(1, 0, 2)

    kwide = np.zeros((128, KW_LEN), np.float32)
    kwide[:, KW_IDENT : KW_IDENT + 128] = np.eye(128, dtype=np.float32)
    kwide[:, KW_QMASK : KW_GSB] = qmask.reshape(128, C6 * NH)
    kwide[:, KW_GSB : KW_LEN] = g_sb.reshape(128, NH * C6 * O)

    in_maps = []
    for core in range(NCORES):
        b0 = BL * core
        hslice = np.ascontiguousarray(hs[b0 : b0 + BL])

        kw = kwide.copy()
        # x0t[p, c*BL + b] = hidden[b0+b, 0, 128c+p]
        kw[:, KW_X0T : KW_QMASK] = (
            hslice[:, 0, :]
            .reshape(BL, C6, 128)
            .transpose(2, 1, 0)
            .reshape(128, C6 * BL)
        )

        in_maps.append(
            {
                "hid": hslice,
                "wq": Wq,
                "wkt": wkt,
                "kwide": kw,
            }
        )
    return in_maps


def kernel(**inputs) -> np.ndarray:
    nc = _get_program()
    in_maps = _host_prep(inputs)
    res = run_bass_kernel_spmd(nc, in_maps, core_ids=list(range(NCORES)))
    return np.concatenate([r["out"] for r in res.results], axis=0).astype(np.float32)


if __name__ == "__main__":
    rng = np.random.default_rng(0)
    demo = {
        "hidden_states": rng.standard_normal((B, S, H), dtype=np.float32),
        "attention_mask": np.ones((B, S), np.float32),
        "Wq": rng.standard_normal((H, H), dtype=np.float32) / np.sqrt(H),
        "bq": np.zeros(H, np.float32),
        "Wk": rng.standard_normal((H, H), dtype=np.float32) / np.sqrt(H),
        "bk": np.zeros(H, np.float32),
        "Wv": rng.standard_normal((H, H), dtype=np.float32) / np.sqrt(H),
        "bv": np.zeros(H, np.float32),
        "Wo": rng.standard_normal((H, O), dtype=np.float32) / np.sqrt(H),
        "bo": np.zeros(O, np.float32),
    }
    out = kernel(**demo)
    print(out.shape, out.dtype)



# revision 16
# speedup vs baseline: 1.3301x; 1.3301x over previous
"""Trainium2 Bass kernel for BERT-style CLS attention head (v2: fp16 dual-layout).

Model (see harness reference):
  q/k/v projections of hidden [B=16, S=1024, H=768], 8 heads x 96,
  softmax attention, but ONLY the CLS token (query position 0) feeds the
  output projection  out = relu(ctx[:, 0] @ Wo + bo)  with Wo [768, 4].

Algebraic structure exploited on-device (per batch b):
  q~      = X[0] @ Wq + bq                  (only row 0 of Q needed)
  Qblk    [768, 16] = diag-blocked q~/sqrt(96)
  Z^T     [16, 768] = Qblk.T @ Wk^T         (K-projection collapses to a
                                             rank-16 op; bk cancels in
                                             softmax)
  scores  [8, 1024] = Z.T @ X^T             (X^T comes pre-transposed
                                             from DRAM - no on-chip
                                             transposes of X at all)
  probs   = exp(scores - 4)                 (unnormalized; shift cancels)
  pt      = probs^T * mask                  (mask folded into the PSUM
                                             evacuation multiply)
  r       [8, 769]  = pt.T @ [X | 1]       (ones-column of X makes
                                             r[:,768] = rowsum -> softmax
                                             denominator for free)
  out     [4]       = relu(sum_hc rt*g/rho + boeff)  (DVE mult-reduce +
                                             one fp32 matmul; G_h =
                                             Wv_h @ Wo_h host-fused)

All streamed tensors are fp16 (host-side cast + layout only; fp32
accumulation in PSUM). Per-core HBM traffic ~8.6 MB -> ~24 us DMA bound.
Sharding: data-parallel over batch, 2 batches per core on 8 cores.
"""

import numpy as np

from concourse import bacc
import concourse.mybir as mybir
import concourse.tile as tile
from concourse.bass import _add_dep_helper
from concourse.bass_utils import run_bass_kernel_spmd

F32 = mybir.dt.float32
import ml_dtypes
NP16 = ml_dtypes.bfloat16
F16 = mybir.dt.bfloat16
F32R = mybir.dt.float32r


def _r(ap):
    return ap.bitcast(F32R)

B, S, H = 16, 1024, 768
NH, DH, O = 8, 96, 4
NCORES = 8
BL = B // NCORES          # 2 batches per core
C6 = H // 128             # 6 hidden chunks of 128
K8 = S // 128             # 8 sequence chunks of 128
HP = 772                  # padded hidden: col 768 = 1.0 (rowsum), 769.. = 0
RCOL = H                  # index of the ones column in padded X

# kw16 packing [128, .] fp16: ident | x0t | g48
KW_IDENT = 0
KW_X0T = 128                       # x0t[p, c*BL + b]
KW_G = KW_X0T + C6 * BL            # 140; g48[p, o*48 + c*NH + h]
KW_ONE = KW_G + O * C6 * NH        # 332: fp16 ones column
KW_LEN = KW_ONE + 4                # 336

# kw32 packing [128, .] fp32: qmask | ones col | boeff (partition 0)
KV_QMASK = 0                       # qmask[p, c*NH + h]
KV_ONES = C6 * NH                  # 48
KV_BOEFF = KV_ONES + 1             # 49 (partition 0 only)
KV_NEG4 = KV_BOEFF + O             # 53: exp bias (-4.0, all partitions)
KV_ID32 = KV_NEG4 + 1              # 54
KV_LEN = KV_ID32 + 128             # 182


def build_program():
    nc = bacc.Bacc(None)

    x_d = nc.declare_dram_parameter("x", [BL, S, HP], F16, isOutput=False)
    xt_d = nc.declare_dram_parameter("xt", [BL, H, S], F16, isOutput=False)
    wq_d = nc.declare_dram_parameter("wq", [H, H], F16, isOutput=False)
    wkt_d = nc.declare_dram_parameter("wkt", [H, H], F16, isOutput=False)
    kw16_d = nc.declare_dram_parameter("kw16", [128, KW_LEN], F16, isOutput=False)
    kw32_d = nc.declare_dram_parameter("kw32", [128, KV_LEN], F32, isOutput=False)
    am_d = nc.declare_dram_parameter("am", [128, BL * K8], F16, isOutput=False)
    bq_d = nc.declare_dram_parameter("bq2", [BL, H], F32, isOutput=False)
    out_d = nc.declare_dram_parameter("out", [BL, O], F32, isOutput=True)

    with tile.TileContext(nc) as tc:
        with (
            tc.tile_pool(name="konst", bufs=1) as kp,
            tc.tile_pool(name="work", bufs=1) as wp,
            tc.tile_pool(name="tps", bufs=2, space="PSUM") as tpsp,
            tc.tile_pool(name="acc", bufs=2, space="PSUM") as accp,
            tc.tile_pool(name="jnk", bufs=1, space="PSUM") as jp,
            tc.tile_pool(name="oup", bufs=1, space="PSUM") as op_,
        ):
            # ---- persistent SBUF tiles ----
            kw16 = kp.tile([128, KW_LEN], F16)
            kw32 = kp.tile([128, KV_LEN], F32)
            am16 = kp.tile([128, BL * K8], F16)
            bq32 = kp.tile([BL, H], F32)
            wq_sb = kp.tile([128, C6, H], F16)
            wkt_sb = kp.tile([128, C6, H], F16)
            x_sb = kp.tile([128, BL, K8, HP], F16)
            xt_sb = kp.tile([128, BL, C6, S], F16)

            ident = kw16[:, KW_IDENT : KW_IDENT + 128]
            x0t_v = kw16[:, KW_X0T : KW_G].rearrange("p (c b) -> p c b", c=C6)
            g48_v = kw16[:, KW_G : KW_ONE].rearrange("p (o a) -> p o a", o=O)
            qmask_v = kw32[:, KV_QMASK : KV_ONES].rearrange("p (c h) -> p c h", c=C6)
            ones_v = kw32[:, KV_ONES : KV_ONES + 1]
            one16_v = kw16[:, KW_ONE : KW_ONE + 1]
            boeff_v = kw32[0:1, KV_BOEFF : KV_BOEFF + O]
            id32 = kw32[:, KV_ID32 : KV_ID32 + 128]
            neg4_v = kw32[:, KV_NEG4 : KV_NEG4 + 1]

            # ---- DMA queue (HWDGE; priority-chained, 2 in flight) ----
            dmas = []
            dmas.append(nc.sync.dma_start(out=kw16[:, :], in_=kw16_d[:, :]))
            dmas.append(nc.sync.dma_start(out=kw32[:, :], in_=kw32_d[:, :]))
            dmas.append(nc.sync.dma_start(out=am16[:, :], in_=am_d[:, :]))
            dmas.append(nc.sync.dma_start(out=bq32[:, :], in_=bq_d[:, :]))
            d_wq = []
            d_wkt = []
            for half, (c0, cn) in enumerate(((0, 3), (3, 6))):
                d_wq.append(
                    nc.sync.dma_start(
                        out=wq_sb[:, c0:cn, :],
                        in_=wq_d.rearrange("(c p) n -> p c n", p=128)[:, c0:cn, :],
                    )
                )
            for half, (c0, cn) in enumerate(((0, 3), (3, 6))):
                d_wkt.append(
                    nc.sync.dma_start(
                        out=wkt_sb[:, c0:cn, :],
                        in_=wkt_d.rearrange("(c p) n -> p c n", p=128)[:, c0:cn, :],
                    )
                )
            dmas.extend(d_wq)
            dmas.extend(d_wkt)

            d_xt = {}   # (b, half) -> dma
            d_x = {}    # (b, piece) -> dma; pieces in s-chunks of 128

            def load_xt(b, nh2):
                d_xt[(b, nh2)] = nc.sync.dma_start(
                    out=xt_sb[:, b, :, 512 * nh2 : 512 * (nh2 + 1)],
                    in_=xt_d[b].rearrange("(c p) s -> p c s", p=128)[
                        :, :, 512 * nh2 : 512 * (nh2 + 1)
                    ],
                )

            def load_x(b, k0, kn):
                d_x[(b, k0)] = nc.sync.dma_start(
                    out=x_sb[:, b, k0:kn, :],
                    in_=x_d[b, 128 * k0 : 128 * kn, :].rearrange(
                        "(k p) i -> p k i", p=128
                    ),
                )

            def load_xt_full(b):
                d_xt[b] = nc.sync.dma_start(
                    out=xt_sb[:, b, :, :],
                    in_=xt_d[b].rearrange("(c p) s -> p c s", p=128),
                )

            load_xt_full(0)
            load_xt_full(1)
            load_x(0, 0, 4)
            load_x(0, 4, 8)
            load_x(1, 0, 4)
            load_x(1, 4, 6)
            load_x(1, 6, 8)
            dmas.extend(
                [
                    d_xt[0], d_xt[1], d_x[(0, 0)], d_x[(0, 4)],
                    d_x[(1, 0)], d_x[(1, 4)], d_x[(1, 6)],
                ]
            )
            # keep two transfers in flight, completing in priority order
            for i in range(2, len(dmas)):
                _add_dep_helper(
                    dmas[i].ins, dmas[i - 2].ins, sync=True, reason="dma order"
                )

            # ---- PE warmup (HAM unthrottle) while weights stream ----
            warm_ps = jp.tile([128, KW_LEN], F32)
            for _ in range(12):
                nc.tensor.matmul(warm_ps[:, :], ident, kw16[:, :])

            # ---- q~ = X[:,0,:] @ Wq + bq : [BL, H] ----
            q_ps = accp.tile([BL, H], F32, tag="acc", name="q_ps")
            for c in range(C6):
                for n0, nw in ((0, 512), (512, 256)):
                    nc.tensor.matmul(
                        q_ps[:, n0 : n0 + nw],
                        x0t_v[:, c, :],
                        wq_sb[:, c, n0 : n0 + nw],
                        start=(c == 0),
                        stop=(c == C6 - 1),
                    )
            q_sb = wp.tile([BL, H], F32)
            nc.vector.tensor_add(q_sb[:, :], q_ps[:, :], bq32[:, :])

            # ---- qT via PE transposes, fused into Qblk = qT * headmask ----
            qt_ps = tpsp.tile([128, 512], F32, tag="tps", name="qt_ps")
            for c in range(C6):
                nc.tensor.transpose(
                    qt_ps[:, BL * c : BL * (c + 1)],
                    q_sb[:, 128 * c : 128 * (c + 1)],
                    id32[:BL, :BL],
                )
            qblk = wp.tile([128, C6, BL, NH], F16)
            nc.vector.tensor_mul(
                qblk[:, :, :, :],
                qt_ps[:, : C6 * BL]
                .rearrange("p (c b) -> p c b", c=C6)
                .unsqueeze(3)
                .to_broadcast([128, C6, BL, NH]),
                qmask_v.unsqueeze(2).to_broadcast([128, C6, BL, NH]),
            )

            # ---- Z^T [16, 768] = Qblk.T @ WkT, then Z [768, 16] ----
            zt_ps = accp.tile([BL * NH, H], F32, tag="acc", name="zt_ps")
            for jc in range(C6):
                for n0, nw in ((0, 512), (512, 256)):
                    nc.tensor.matmul(
                        zt_ps[:, n0 : n0 + nw],
                        qblk[:, jc, :, :].rearrange("p b h -> p (b h)"),
                        wkt_sb[:, jc, n0 : n0 + nw],
                        start=(jc == 0),
                        stop=(jc == C6 - 1),
                    )
            zt_sb = wp.tile([BL * NH, H], F32)
            nc.vector.tensor_copy(zt_sb[:, :], zt_ps[:, :])
            z_tps = tpsp.tile([128, 512], F32, tag="tps", name="z_tps")
            for c in range(C6):
                nc.tensor.transpose(
                    z_tps[:, 16 * c : 16 * (c + 1)],
                    zt_sb[:, 128 * c : 128 * (c + 1)],
                    id32[: BL * NH, : BL * NH],
                )
            z_sb = wp.tile([128, C6, BL * NH], F16)
            nc.vector.tensor_copy(z_sb[:, :, :], z_tps[:, : C6 * BL * NH].rearrange("p (c a) -> p c a", c=C6))

            # ---- per-batch attention pipeline ----
            probs = wp.tile([NH, BL, S], F32)
            pt_sb = wp.tile([128, BL, K8, NH], F16)

            def scores_half(b, sc_ps, nh2):
                for ic in range(C6):
                    nc.tensor.matmul(
                        sc_ps[:, 512 * nh2 : 512 * (nh2 + 1)],
                        z_sb[:, ic, NH * b : NH * (b + 1)],
                        xt_sb[:, b, ic, 512 * nh2 : 512 * (nh2 + 1)],
                        start=(ic == 0),
                        stop=(ic == C6 - 1),
                    )

            def exp_half(b, sc_ps, nh2):
                # shift-invariant exp; -4 guards fp16 range (|score| <~ 7)
                nc.scalar.activation(
                    probs[:, b, 512 * nh2 : 512 * (nh2 + 1)],
                    sc_ps[:, 512 * nh2 : 512 * (nh2 + 1)],
                    mybir.ActivationFunctionType.Exp,
                    bias=neg4_v[:NH, :],
                    scale=1.0,
                )

            def pt_block(b):
                pt_ps = tpsp.tile([128, 512], F32, tag="tps", name=f"pt_ps{b}")
                for k in range(K8):
                    nc.tensor.transpose(
                        pt_ps[:, NH * k : NH * (k + 1)],
                        probs[:, b, 128 * k : 128 * (k + 1)],
                        id32[:NH, :NH],
                    )
                # attention mask folded into the PSUM evacuation (exact:
                # exp(score - 10000) == 0 == exp(score) * mask in fp32)
                nc.vector.tensor_mul(
                    pt_sb[:, b, :, :],
                    pt_ps[:, : K8 * NH].rearrange("p (k h) -> p k h", k=K8),
                    am16[:, b * K8 : (b + 1) * K8]
                    .unsqueeze(2)
                    .to_broadcast([128, K8, NH]),
                )

            def r_chunks(b, r_ps, k0, kn):
                for k in range(k0, kn):
                    for n0, nw in ((0, 512), (512, HP - 512)):
                        nc.tensor.matmul(
                            r_ps[:, n0 : n0 + nw],
                            pt_sb[:, b, k, :],
                            x_sb[:, b, k, n0 : n0 + nw],
                            start=(k == 0),
                            stop=(k == K8 - 1),
                        )

            def finish(b, r_ps):
                recip = wp.tile([NH, 1], F32, name=f"recip{b}")
                nc.vector.reciprocal(recip[:, :], r_ps[:, RCOL : RCOL + 1])
                rsc = wp.tile([NH, H], F32, name=f"rsc{b}")
                nc.vector.tensor_scalar_mul(
                    rsc[:, :], r_ps[:, :H], recip[:, :]
                )
                rt_ps = tpsp.tile([128, 512], F32, tag="tps", name=f"rt_ps{b}")
                for c in range(C6):
                    nc.tensor.transpose(
                        rt_ps[:, NH * c : NH * (c + 1)],
                        rsc[:, 128 * c : 128 * (c + 1)],
                        id32[:NH, :NH],
                    )
                rt_sb = wp.tile([128, C6 * NH], F16, name=f"rt_sb{b}")
                nc.vector.tensor_copy(rt_sb[:, :], rt_ps[:, : C6 * NH])
                scrap = wp.tile([128, O, C6 * NH], F16, name=f"scrap{b}")
                partials = wp.tile([128, O], F32, name=f"partials{b}")
                for o in range(O):
                    nc.vector.tensor_mul(
                        scrap[:, o, :], rt_sb[:, :], g48_v[:, o, :]
                    )
                    nc.vector.tensor_reduce(
                        partials[:, o : o + 1],
                        scrap[:, o, :],
                        mybir.AxisListType.X,
                        mybir.AluOpType.add,
                    )
                partials16 = wp.tile([128, O], F16, name=f"partials16{b}")
                nc.vector.tensor_copy(partials16[:, :], partials[:, :])
                osum_ps = op_.tile([1, O], F32, tag="out", name=f"osum{b}")
                nc.tensor.matmul(
                    osum_ps[:, :], one16_v, partials16[:, :], start=True, stop=True
                )
                out1 = wp.tile([1, O], F32, name=f"out1{b}")
                nc.vector.tensor_add(out1[:, :], osum_ps[:, :], boeff_v)
                out_sb = wp.tile([1, O], F32, name=f"out_sb{b}")
                nc.vector.tensor_scalar_max(out_sb[:, :], out1[:, :], 0.0)
                nc.sync.dma_start(out=out_d[b : b + 1, :], in_=out_sb[:, :])

            # interleaved two-batch pipeline: PE chases the DMA queue
            sc_ps0 = accp.tile([NH, S], F32, tag="acc", name="sc_ps0")
            scores_half(0, sc_ps0, 0)
            scores_half(0, sc_ps0, 1)
            exp_half(0, sc_ps0, 0)
            exp_half(0, sc_ps0, 1)
            pt_block(0)
            sc_ps1 = accp.tile([NH, S], F32, tag="acc", name="sc_ps1")
            scores_half(1, sc_ps1, 0)
            scores_half(1, sc_ps1, 1)
            exp_half(1, sc_ps1, 0)
            exp_half(1, sc_ps1, 1)
            r_ps0 = accp.tile([NH, HP], F32, tag="acc", name="r_ps0")
            r_chunks(0, r_ps0, 0, 4)
            pt_block(1)
            r_chunks(0, r_ps0, 4, 8)
            finish(0, r_ps0)
            r_ps1 = accp.tile([NH, HP], F32, tag="acc", name="r_ps1")
            r_chunks(1, r_ps1, 0, 4)
            r_chunks(1, r_ps1, 4, 6)
            r_chunks(1, r_ps1, 6, 8)
            finish(1, r_ps1)

    nc.finalize()
    return nc


_NC_CACHE = None


def _get_program():
    global _NC_CACHE
    if _NC_CACHE is None:
        _NC_CACHE = build_program()
    return _NC_CACHE


def _host_prep(inputs):
    """Weight fusion + fp16 cast + layout prep (host side)."""
    hs = np.asarray(inputs["hidden_states"], np.float32)
    am = np.asarray(inputs["attention_mask"], np.float32)
    Wq = np.asarray(inputs["Wq"], np.float32)
    bq = np.asarray(inputs["bq"], np.float32)
    Wk = np.asarray(inputs["Wk"], np.float32)
    Wv = np.asarray(inputs["Wv"], np.float32)
    bv = np.asarray(inputs["bv"], np.float32)
    Wo = np.asarray(inputs["Wo"], np.float32)
    bo = np.asarray(inputs["bo"], np.float32)

    wq16 = np.ascontiguousarray(Wq.astype(NP16))
    wkt16 = np.ascontiguousarray(Wk.T.astype(NP16))

    # g48[p, o, c*8+h] = G_h[128c+p, o],  G_h = Wv[:, h] @ Wo[h, :]
    g48 = np.zeros((128, O, C6 * NH), NP16)
    for h in range(NH):
        Gh = (Wv[:, DH * h : DH * (h + 1)] @ Wo[DH * h : DH * (h + 1), :]).astype(
            NP16
        )
        for c in range(C6):
            g48[:, :, c * NH + h] = Gh[128 * c : 128 * (c + 1), :]

    boeff = (bo + bv @ Wo).astype(np.float32)

    # qmask[p, c*8+h]: 1/sqrt(96) where hidden index 128c+p is in head h
    j = np.arange(H)
    qm = np.zeros((H, NH), np.float32)
    qm[j, j // DH] = 1.0 / np.sqrt(np.float32(DH))
    qm = qm.reshape(C6, 128, NH).transpose(1, 0, 2).reshape(128, C6 * NH)

    kw32 = np.zeros((128, KV_LEN), np.float32)
    kw32[:, KV_QMASK:KV_ONES] = qm
    kw32[:, KV_ONES] = 1.0
    kw32[0, KV_BOEFF : KV_BOEFF + O] = boeff
    kw32[:, KV_NEG4] = -4.0
    kw32[:, KV_ID32 : KV_ID32 + 128] = np.eye(128, dtype=np.float32)

    kw16_base = np.zeros((128, KW_LEN), NP16)
    kw16_base[:, KW_IDENT : KW_IDENT + 128] = np.eye(128, dtype=NP16)
    kw16_base[:, KW_G:KW_ONE] = g48.reshape(128, O * C6 * NH)
    kw16_base[:, KW_ONE] = 1.0

    bq2 = np.broadcast_to(bq, (BL, H)).astype(np.float32).copy()

    hs16 = hs.astype(NP16)

    in_maps = []
    for core in range(NCORES):
        b0 = BL * core
        xpad = np.zeros((BL, S, HP), NP16)
        xpad[:, :, :H] = hs16[b0 : b0 + BL]
        xpad[:, :, RCOL] = 1.0
        xt = np.ascontiguousarray(hs16[b0 : b0 + BL].transpose(0, 2, 1))

        kw16 = kw16_base.copy()
        # x0t[p, c*BL+b] = X[b0+b, 0, 128c+p]
        kw16[:, KW_X0T:KW_G] = (
            hs16[b0 : b0 + BL, 0, :]
            .reshape(BL, C6, 128)
            .transpose(2, 1, 0)
            .reshape(128, C6 * BL)
        )

        # am[p, b*K8+k] = mask[b0+b, 128k+p]
        amc = (
            am[b0 : b0 + BL, :]
            .reshape(BL, K8, 128)
            .transpose(2, 0, 1)
            .reshape(128, BL * K8)
            .astype(NP16)
        )

        in_maps.append(
            {
                "x": xpad,
                "xt": xt,
                "wq": wq16,
                "wkt": wkt16,
                "kw16": kw16,
                "kw32": kw32,
                "am": np.ascontiguousarray(amc),
                "bq2": bq2,
            }
        )
    return in_maps


def kernel(**inputs) -> np.ndarray:
    nc = _get_program()
    in_maps = _host_prep(inputs)
    res = run_bass_kernel_spmd(nc, in_maps, core_ids=list(range(NCORES)))
    return np.concatenate([r["out"] for r in res.results], axis=0).astype(np.float32)


if __name__ == "__main__":
    rng = np.random.default_rng(0)
    demo = {
        "hidden_states": rng.standard_normal((B, S, H), dtype=np.float32),
        "attention_mask": np.ones((B, S), np.float32),
        "Wq": rng.standard_normal((H, H), dtype=np.float32) / np.sqrt(H),
        "bq": np.zeros(H, np.float32),
        "Wk": rng.standard_normal((H, H), dtype=np.float32) / np.sqrt(H),
        "bk": np.zeros(H, np.float32),
        "Wv": rng.standard_normal((H, H), dtype=np.float32) / np.sqrt(H),
        "bv": np.zeros(H, np.float32),
        "Wo": rng.standard_normal((H, O), dtype=np.float32) / np.sqrt(H),
        "bo": np.zeros(O, np.float32),
    }
    out = kernel(**demo)
    print(out.shape, out.dtype)


# revision 17
# speedup vs baseline: 1.4363x; 1.0798x over previous
"""Trainium2 Bass kernel for BERT-style CLS attention head (v2: fp16 dual-layout).

Model (see harness reference):
  q/k/v projections of hidden [B=16, S=1024, H=768], 8 heads x 96,
  softmax attention, but ONLY the CLS token (query position 0) feeds the
  output projection  out = relu(ctx[:, 0] @ Wo + bo)  with Wo [768, 4].

Algebraic structure exploited on-device (per batch b):
  q~      = X[0] @ Wq + bq                  (only row 0 of Q needed)
  Qblk    [768, 16] = diag-blocked q~/sqrt(96)
  Z^T     [16, 768] = Qblk.T @ Wk^T         (K-projection collapses to a
                                             rank-16 op; bk cancels in
                                             softmax)
  scores  [8, 1024] = Z.T @ X^T             (X^T comes pre-transposed
                                             from DRAM - no on-chip
                                             transposes of X at all)
  probs   = exp(scores - 4)                 (unnormalized; shift cancels)
  pt      = probs^T * mask                  (mask folded into the PSUM
                                             evacuation multiply)
  r       [8, 769]  = pt.T @ [X | 1]       (ones-column of X makes
                                             r[:,768] = rowsum -> softmax
                                             denominator for free)
  out     [4]       = relu(sum_hc rt*g/rho + boeff)  (DVE mult-reduce +
                                             one fp32 matmul; G_h =
                                             Wv_h @ Wo_h host-fused)

All streamed tensors are fp16 (host-side cast + layout only; fp32
accumulation in PSUM). Per-core HBM traffic ~8.6 MB -> ~24 us DMA bound.
Sharding: data-parallel over batch, 2 batches per core on 8 cores.
"""

import numpy as np

from concourse import bacc
import concourse.mybir as mybir
import concourse.tile as tile
from concourse.bass import _add_dep_helper
from concourse.bass_utils import run_bass_kernel_spmd

F32 = mybir.dt.float32
import ml_dtypes
NP16 = ml_dtypes.bfloat16
F16 = mybir.dt.bfloat16
F32R = mybir.dt.float32r


def _r(ap):
    return ap.bitcast(F32R)

B, S, H = 16, 1024, 768
NH, DH, O = 8, 96, 4
NCORES = 8
BL = B // NCORES          # 2 batches per core
C6 = H // 128             # 6 hidden chunks of 128
K8 = S // 128             # 8 sequence chunks of 128
HP = 772                  # padded hidden: col 768 = 1.0 (rowsum), 769.. = 0
RCOL = H                  # index of the ones column in padded X

# kw16 packing [128, .] fp16: ident | x0t | g48
KW_IDENT = 0
KW_X0T = 128                       # x0t[p, c*BL + b]
KW_G = KW_X0T + C6 * BL            # 140; g48[p, o*48 + c*NH + h]
KW_ONE = KW_G + O * C6 * NH        # 332: fp16 ones column
KW_LEN = KW_ONE + 4                # 336

# kw32 packing [128, .] fp32: qmask | ones col | boeff (partition 0)
KV_QMASK = 0                       # qmask[p, c*NH + h]
KV_ONES = C6 * NH                  # 48
KV_BOEFF = KV_ONES + 1             # 49 (partition 0 only)
KV_NEG4 = KV_BOEFF + O             # 53: exp bias (-4.0, all partitions)
KV_ID32 = KV_NEG4 + 1              # 54
KV_LEN = KV_ID32 + 128             # 182


def build_program():
    nc = bacc.Bacc(None)

    x_d = nc.declare_dram_parameter("x", [BL, S, HP], F16, isOutput=False)
    xt_d = nc.declare_dram_parameter("xt", [BL, H, S], F16, isOutput=False)
    wq_d = nc.declare_dram_parameter("wq", [H, H], F16, isOutput=False)
    wkt_d = nc.declare_dram_parameter("wkt", [H, H], F16, isOutput=False)
    kw16_d = nc.declare_dram_parameter("kw16", [128, KW_LEN], F16, isOutput=False)
    kw32_d = nc.declare_dram_parameter("kw32", [128, KV_LEN], F32, isOutput=False)
    am_d = nc.declare_dram_parameter("am", [128, BL * K8], F16, isOutput=False)
    bq_d = nc.declare_dram_parameter("bq2", [BL, H], F32, isOutput=False)
    out_d = nc.declare_dram_parameter("out", [BL, O], F32, isOutput=True)

    with tile.TileContext(nc) as tc:
        with (
            tc.tile_pool(name="konst", bufs=1) as kp,
            tc.tile_pool(name="work", bufs=1) as wp,
            tc.tile_pool(name="tps", bufs=2, space="PSUM") as tpsp,
            tc.tile_pool(name="acc", bufs=2, space="PSUM") as accp,
            tc.tile_pool(name="jnk", bufs=1, space="PSUM") as jp,
            tc.tile_pool(name="oup", bufs=1, space="PSUM") as op_,
        ):
            # ---- persistent SBUF tiles ----
            kw16 = kp.tile([128, KW_LEN], F16)
            kw32 = kp.tile([128, KV_LEN], F32)
            am16 = kp.tile([128, BL * K8], F16)
            bq32 = kp.tile([BL, H], F32)
            wq_sb = kp.tile([128, C6, H], F16)
            wkt_sb = kp.tile([128, C6, H], F16)
            x_sb = kp.tile([128, BL, K8, HP], F16)
            xt_sb = kp.tile([128, BL, C6, S], F16)

            ident = kw16[:, KW_IDENT : KW_IDENT + 128]
            x0t_v = kw16[:, KW_X0T : KW_G].rearrange("p (c b) -> p c b", c=C6)
            g48_v = kw16[:, KW_G : KW_ONE].rearrange("p (o a) -> p o a", o=O)
            qmask_v = kw32[:, KV_QMASK : KV_ONES].rearrange("p (c h) -> p c h", c=C6)
            ones_v = kw32[:, KV_ONES : KV_ONES + 1]
            one16_v = kw16[:, KW_ONE : KW_ONE + 1]
            boeff_v = kw32[0:1, KV_BOEFF : KV_BOEFF + O]
            id32 = kw32[:, KV_ID32 : KV_ID32 + 128]
            neg4_v = kw32[:, KV_NEG4 : KV_NEG4 + 1]

            # ---- DMA queue (HWDGE; priority-chained, 2 in flight) ----
            dmas = []
            dmas.append(nc.sync.dma_start(out=kw16[:, :], in_=kw16_d[:, :]))
            dmas.append(nc.sync.dma_start(out=kw32[:, :], in_=kw32_d[:, :]))
            dmas.append(nc.sync.dma_start(out=am16[:, :], in_=am_d[:, :]))
            dmas.append(nc.sync.dma_start(out=bq32[:, :], in_=bq_d[:, :]))
            d_wq = []
            d_wkt = []
            for half, (c0, cn) in enumerate(((0, 3), (3, 6))):
                d_wq.append(
                    nc.sync.dma_start(
                        out=wq_sb[:, c0:cn, :],
                        in_=wq_d.rearrange("(c p) n -> p c n", p=128)[:, c0:cn, :],
                    )
                )
            for half, (c0, cn) in enumerate(((0, 3), (3, 6))):
                d_wkt.append(
                    nc.sync.dma_start(
                        out=wkt_sb[:, c0:cn, :],
                        in_=wkt_d.rearrange("(c p) n -> p c n", p=128)[:, c0:cn, :],
                    )
                )
            dmas.extend(d_wq)
            dmas.extend(d_wkt)

            d_xt = {}   # (b, half) -> dma
            d_x = {}    # (b, piece) -> dma; pieces in s-chunks of 128

            def load_xt(b, nh2):
                d_xt[(b, nh2)] = nc.sync.dma_start(
                    out=xt_sb[:, b, :, 512 * nh2 : 512 * (nh2 + 1)],
                    in_=xt_d[b].rearrange("(c p) s -> p c s", p=128)[
                        :, :, 512 * nh2 : 512 * (nh2 + 1)
                    ],
                )

            def load_x(b, k0, kn):
                d_x[(b, k0)] = nc.sync.dma_start(
                    out=x_sb[:, b, k0:kn, :],
                    in_=x_d[b, 128 * k0 : 128 * kn, :].rearrange(
                        "(k p) i -> p k i", p=128
                    ),
                )

            def load_xt_full(b):
                d_xt[b] = nc.sync.dma_start(
                    out=xt_sb[:, b, :, :],
                    in_=xt_d[b].rearrange("(c p) s -> p c s", p=128),
                )

            load_xt_full(0)
            load_xt_full(1)
            load_x(0, 0, 4)
            load_x(0, 4, 8)
            load_x(1, 0, 4)
            load_x(1, 4, 6)
            load_x(1, 6, 8)
            dmas.extend(
                [
                    d_xt[0], d_xt[1], d_x[(0, 0)], d_x[(0, 4)],
                    d_x[(1, 0)], d_x[(1, 4)], d_x[(1, 6)],
                ]
            )
            # keep four transfers in flight: hides the ~2.2us per-transfer
            # HWDGE gen + completion-receipt latency while the single ring's
            # FIFO drain keeps bytes landing in priority order
            for i in range(4, len(dmas)):
                _add_dep_helper(
                    dmas[i].ins, dmas[i - 4].ins, sync=True, reason="dma order"
                )

            # ---- PE warmup (HAM unthrottle) while weights stream ----
            warm_ps = jp.tile([128, KW_LEN], F32)
            for _ in range(12):
                nc.tensor.matmul(warm_ps[:, :], ident, kw16[:, :])

            # ---- q~ = X[:,0,:] @ Wq + bq : [BL, H] ----
            q_ps = accp.tile([BL, H], F32, tag="acc", name="q_ps")
            for c in range(C6):
                for n0, nw in ((0, 512), (512, 256)):
                    nc.tensor.matmul(
                        q_ps[:, n0 : n0 + nw],
                        x0t_v[:, c, :],
                        wq_sb[:, c, n0 : n0 + nw],
                        start=(c == 0),
                        stop=(c == C6 - 1),
                    )
            q_sb = wp.tile([BL, H], F32)
            nc.vector.tensor_add(q_sb[:, :], q_ps[:, :], bq32[:, :])

            # ---- qT via PE transposes, fused into Qblk = qT * headmask ----
            qt_ps = tpsp.tile([128, 512], F32, tag="tps", name="qt_ps")
            for c in range(C6):
                nc.tensor.transpose(
                    qt_ps[:, BL * c : BL * (c + 1)],
                    q_sb[:, 128 * c : 128 * (c + 1)],
                    id32[:BL, :BL],
                )
            qblk = wp.tile([128, C6, BL, NH], F16)
            nc.vector.tensor_mul(
                qblk[:, :, :, :],
                qt_ps[:, : C6 * BL]
                .rearrange("p (c b) -> p c b", c=C6)
                .unsqueeze(3)
                .to_broadcast([128, C6, BL, NH]),
                qmask_v.unsqueeze(2).to_broadcast([128, C6, BL, NH]),
            )

            # ---- Z^T [16, 768] = Qblk.T @ WkT, then Z [768, 16] ----
            zt_ps = accp.tile([BL * NH, H], F32, tag="acc", name="zt_ps")
            for jc in range(C6):
                for n0, nw in ((0, 512), (512, 256)):
                    nc.tensor.matmul(
                        zt_ps[:, n0 : n0 + nw],
                        qblk[:, jc, :, :].rearrange("p b h -> p (b h)"),
                        wkt_sb[:, jc, n0 : n0 + nw],
                        start=(jc == 0),
                        stop=(jc == C6 - 1),
                    )
            zt_sb = wp.tile([BL * NH, H], F32)
            nc.vector.tensor_copy(zt_sb[:, :], zt_ps[:, :])
            z_tps = tpsp.tile([128, 512], F32, tag="tps", name="z_tps")
            for c in range(C6):
                nc.tensor.transpose(
                    z_tps[:, 16 * c : 16 * (c + 1)],
                    zt_sb[:, 128 * c : 128 * (c + 1)],
                    id32[: BL * NH, : BL * NH],
                )
            z_sb = wp.tile([128, C6, BL * NH], F16)
            nc.vector.tensor_copy(z_sb[:, :, :], z_tps[:, : C6 * BL * NH].rearrange("p (c a) -> p c a", c=C6))

            # ---- per-batch attention pipeline ----
            probs = wp.tile([NH, BL, S], F32)
            pt_sb = wp.tile([128, BL, K8, NH], F16)

            def scores_half(b, sc_ps, nh2):
                for ic in range(C6):
                    nc.tensor.matmul(
                        sc_ps[:, 512 * nh2 : 512 * (nh2 + 1)],
                        z_sb[:, ic, NH * b : NH * (b + 1)],
                        xt_sb[:, b, ic, 512 * nh2 : 512 * (nh2 + 1)],
                        start=(ic == 0),
                        stop=(ic == C6 - 1),
                    )

            def exp_half(b, sc_ps, nh2):
                # shift-invariant exp; -4 guards fp16 range (|score| <~ 7)
                nc.scalar.activation(
                    probs[:, b, 512 * nh2 : 512 * (nh2 + 1)],
                    sc_ps[:, 512 * nh2 : 512 * (nh2 + 1)],
                    mybir.ActivationFunctionType.Exp,
                    bias=neg4_v[:NH, :],
                    scale=1.0,
                )

            def pt_block(b):
                pt_ps = tpsp.tile([128, 512], F32, tag="tps", name=f"pt_ps{b}")
                for k in range(K8):
                    nc.tensor.transpose(
                        pt_ps[:, NH * k : NH * (k + 1)],
                        probs[:, b, 128 * k : 128 * (k + 1)],
                        id32[:NH, :NH],
                    )
                # attention mask folded into the PSUM evacuation (exact:
                # exp(score - 10000) == 0 == exp(score) * mask in fp32)
                nc.vector.tensor_mul(
                    pt_sb[:, b, :, :],
                    pt_ps[:, : K8 * NH].rearrange("p (k h) -> p k h", k=K8),
                    am16[:, b * K8 : (b + 1) * K8]
                    .unsqueeze(2)
                    .to_broadcast([128, K8, NH]),
                )

            def r_chunks(b, r_ps, k0, kn):
                for k in range(k0, kn):
                    for n0, nw in ((0, 512), (512, HP - 512)):
                        nc.tensor.matmul(
                            r_ps[:, n0 : n0 + nw],
                            pt_sb[:, b, k, :],
                            x_sb[:, b, k, n0 : n0 + nw],
                            start=(k == 0),
                            stop=(k == K8 - 1),
                        )

            def finish(b, r_ps):
                recip = wp.tile([NH, 1], F32, name=f"recip{b}")
                nc.vector.reciprocal(recip[:, :], r_ps[:, RCOL : RCOL + 1])
                rsc = wp.tile([NH, H], F32, name=f"rsc{b}")
                nc.vector.tensor_scalar_mul(
                    rsc[:, :], r_ps[:, :H], recip[:, :]
                )
                rt_ps = tpsp.tile([128, 512], F32, tag="tps", name=f"rt_ps{b}")
                for c in range(C6):
                    nc.tensor.transpose(
                        rt_ps[:, NH * c : NH * (c + 1)],
                        rsc[:, 128 * c : 128 * (c + 1)],
                        id32[:NH, :NH],
                    )
                rt_sb = wp.tile([128, C6 * NH], F16, name=f"rt_sb{b}")
                nc.vector.tensor_copy(rt_sb[:, :], rt_ps[:, : C6 * NH])
                scrap = wp.tile([128, O, C6 * NH], F16, name=f"scrap{b}")
                partials = wp.tile([128, O], F32, name=f"partials{b}")
                for o in range(O):
                    nc.vector.tensor_mul(
                        scrap[:, o, :], rt_sb[:, :], g48_v[:, o, :]
                    )
                    nc.vector.tensor_reduce(
                        partials[:, o : o + 1],
                        scrap[:, o, :],
                        mybir.AxisListType.X,
                        mybir.AluOpType.add,
                    )
                partials16 = wp.tile([128, O], F16, name=f"partials16{b}")
                nc.vector.tensor_copy(partials16[:, :], partials[:, :])
                osum_ps = op_.tile([1, O], F32, tag="out", name=f"osum{b}")
                nc.tensor.matmul(
                    osum_ps[:, :], one16_v, partials16[:, :], start=True, stop=True
                )
                out1 = wp.tile([1, O], F32, name=f"out1{b}")
                nc.vector.tensor_add(out1[:, :], osum_ps[:, :], boeff_v)
                out_sb = wp.tile([1, O], F32, name=f"out_sb{b}")
                nc.vector.tensor_scalar_max(out_sb[:, :], out1[:, :], 0.0)
                nc.sync.dma_start(out=out_d[b : b + 1, :], in_=out_sb[:, :])

            # interleaved two-batch pipeline: PE chases the DMA queue
            sc_ps0 = accp.tile([NH, S], F32, tag="acc", name="sc_ps0")
            scores_half(0, sc_ps0, 0)
            scores_half(0, sc_ps0, 1)
            exp_half(0, sc_ps0, 0)
            exp_half(0, sc_ps0, 1)
            pt_block(0)
            sc_ps1 = accp.tile([NH, S], F32, tag="acc", name="sc_ps1")
            scores_half(1, sc_ps1, 0)
            scores_half(1, sc_ps1, 1)
            exp_half(1, sc_ps1, 0)
            exp_half(1, sc_ps1, 1)
            r_ps0 = accp.tile([NH, HP], F32, tag="acc", name="r_ps0")
            r_chunks(0, r_ps0, 0, 4)
            pt_block(1)
            r_chunks(0, r_ps0, 4, 8)
            finish(0, r_ps0)
            r_ps1 = accp.tile([NH, HP], F32, tag="acc", name="r_ps1")
            r_chunks(1, r_ps1, 0, 4)
            r_chunks(1, r_ps1, 4, 6)
            r_chunks(1, r_ps1, 6, 8)
            finish(1, r_ps1)

    nc.finalize()
    return nc


_NC_CACHE = None


def _get_program():
    global _NC_CACHE
    if _NC_CACHE is None:
        _NC_CACHE = build_program()
    return _NC_CACHE


def _host_prep(inputs):
    """Weight fusion + fp16 cast + layout prep (host side)."""
    hs = np.asarray(inputs["hidden_states"], np.float32)
    am = np.asarray(inputs["attention_mask"], np.float32)
    Wq = np.asarray(inputs["Wq"], np.float32)
    bq = np.asarray(inputs["bq"], np.float32)
    Wk = np.asarray(inputs["Wk"], np.float32)
    Wv = np.asarray(inputs["Wv"], np.float32)
    bv = np.asarray(inputs["bv"], np.float32)
    Wo = np.asarray(inputs["Wo"], np.float32)
    bo = np.asarray(inputs["bo"], np.float32)

    wq16 = np.ascontiguousarray(Wq.astype(NP16))
    wkt16 = np.ascontiguousarray(Wk.T.astype(NP16))

    # g48[p, o, c*8+h] = G_h[128c+p, o],  G_h = Wv[:, h] @ Wo[h, :]
    g48 = np.zeros((128, O, C6 * NH), NP16)
    for h in range(NH):
        Gh = (Wv[:, DH * h : DH * (h + 1)] @ Wo[DH * h : DH * (h + 1), :]).astype(
            NP16
        )
        for c in range(C6):
            g48[:, :, c * NH + h] = Gh[128 * c : 128 * (c + 1), :]

    boeff = (bo + bv @ Wo).astype(np.float32)

    # qmask[p, c*8+h]: 1/sqrt(96) where hidden index 128c+p is in head h
    j = np.arange(H)
    qm = np.zeros((H, NH), np.float32)
    qm[j, j // DH] = 1.0 / np.sqrt(np.float32(DH))
    qm = qm.reshape(C6, 128, NH).transpose(1, 0, 2).reshape(128, C6 * NH)

    kw32 = np.zeros((128, KV_LEN), np.float32)
    kw32[:, KV_QMASK:KV_ONES] = qm
    kw32[:, KV_ONES] = 1.0
    kw32[0, KV_BOEFF : KV_BOEFF + O] = boeff
    kw32[:, KV_NEG4] = -4.0
    kw32[:, KV_ID32 : KV_ID32 + 128] = np.eye(128, dtype=np.float32)

    kw16_base = np.zeros((128, KW_LEN), NP16)
    kw16_base[:, KW_IDENT : KW_IDENT + 128] = np.eye(128, dtype=NP16)
    kw16_base[:, KW_G:KW_ONE] = g48.reshape(128, O * C6 * NH)
    kw16_base[:, KW_ONE] = 1.0

    bq2 = np.broadcast_to(bq, (BL, H)).astype(np.float32).copy()

    hs16 = hs.astype(NP16)

    in_maps = []
    for core in range(NCORES):
        b0 = BL * core
        xpad = np.zeros((BL, S, HP), NP16)
        xpad[:, :, :H] = hs16[b0 : b0 + BL]
        xpad[:, :, RCOL] = 1.0
        xt = np.ascontiguousarray(hs16[b0 : b0 + BL].transpose(0, 2, 1))

        kw16 = kw16_base.copy()
        # x0t[p, c*BL+b] = X[b0+b, 0, 128c+p]
        kw16[:, KW_X0T:KW_G] = (
            hs16[b0 : b0 + BL, 0, :]
            .reshape(BL, C6, 128)
            .transpose(2, 1, 0)
            .reshape(128, C6 * BL)
        )

        # am[p, b*K8+k] = mask[b0+b, 128k+p]
        amc = (
            am[b0 : b0 + BL, :]
            .reshape(BL, K8, 128)
            .transpose(2, 0, 1)
            .reshape(128, BL * K8)
            .astype(NP16)
        )

        in_maps.append(
            {
                "x": xpad,
                "xt": xt,
                "wq": wq16,
                "wkt": wkt16,
                "kw16": kw16,
                "kw32": kw32,
                "am": np.ascontiguousarray(amc),
                "bq2": bq2,
            }
        )
    return in_maps


def kernel(**inputs) -> np.ndarray:
    nc = _get_program()
    in_maps = _host_prep(inputs)
    res = run_bass_kernel_spmd(nc, in_maps, core_ids=list(range(NCORES)))
    return np.concatenate([r["out"] for r in res.results], axis=0).astype(np.float32)


if __name__ == "__main__":
    rng = np.random.default_rng(0)
    demo = {
        "hidden_states": rng.standard_normal((B, S, H), dtype=np.float32),
        "attention_mask": np.ones((B, S), np.float32),
        "Wq": rng.standard_normal((H, H), dtype=np.float32) / np.sqrt(H),
        "bq": np.zeros(H, np.float32),
        "Wk": rng.standard_normal((H, H), dtype=np.float32) / np.sqrt(H),
        "bk": np.zeros(H, np.float32),
        "Wv": rng.standard_normal((H, H), dtype=np.float32) / np.sqrt(H),
        "bv": np.zeros(H, np.float32),
        "Wo": rng.standard_normal((H, O), dtype=np.float32) / np.sqrt(H),
        "bo": np.zeros(O, np.float32),
    }
    out = kernel(**demo)
    print(out.shape, out.dtype)


# revision 18
# speedup vs baseline: 1.6198x; 1.1278x over previous
"""Trainium2 Bass kernel for BERT-style CLS attention head (v2: fp16 dual-layout).

Model (see harness reference):
  q/k/v projections of hidden [B=16, S=1024, H=768], 8 heads x 96,
  softmax attention, but ONLY the CLS token (query position 0) feeds the
  output projection  out = relu(ctx[:, 0] @ Wo + bo)  with Wo [768, 4].

Algebraic structure exploited on-device (per batch b):
  q~      = X[0] @ Wq + bq                  (only row 0 of Q needed)
  Qblk    [768, 16] = diag-blocked q~/sqrt(96)
  Z^T     [16, 768] = Qblk.T @ Wk^T         (K-projection collapses to a
                                             rank-16 op; bk cancels in
                                             softmax)
  scores  [8, 1024] = Z.T @ X^T             (X^T comes pre-transposed
                                             from DRAM - no on-chip
                                             transposes of X at all)
  probs   = exp(scores - 4)                 (unnormalized; shift cancels)
  pt      = probs^T * mask                  (mask folded into the PSUM
                                             evacuation multiply)
  r       [8, 769]  = pt.T @ [X | 1]       (ones-column of X makes
                                             r[:,768] = rowsum -> softmax
                                             denominator for free)
  out     [4]       = relu(sum_hc rt*g/rho + boeff)  (DVE mult-reduce +
                                             one fp32 matmul; G_h =
                                             Wv_h @ Wo_h host-fused)

All streamed tensors are fp16 (host-side cast + layout only; fp32
accumulation in PSUM). Per-core HBM traffic ~8.6 MB -> ~24 us DMA bound.
Sharding: data-parallel over batch, 2 batches per core on 8 cores.
"""

import numpy as np

from concourse import bacc
import concourse.mybir as mybir
import concourse.tile as tile
from concourse.bass import _add_dep_helper
from concourse.bass_utils import run_bass_kernel_spmd

F32 = mybir.dt.float32
import ml_dtypes
NP16 = ml_dtypes.bfloat16
F16 = mybir.dt.bfloat16
F32R = mybir.dt.float32r


def _r(ap):
    return ap.bitcast(F32R)

B, S, H = 16, 1024, 768
NH, DH, O = 8, 96, 4
NCORES = 8
BL = B // NCORES          # 2 batches per core
C6 = H // 128             # 6 hidden chunks of 128
K8 = S // 128             # 8 sequence chunks of 128
HP = 772                  # padded hidden: col 768 = 1.0 (rowsum), 769.. = 0
RCOL = H                  # index of the ones column in padded X

# kw16 packing [128, .] fp16: ident | x0t | g48
KW_IDENT = 0
KW_X0T = 128                       # x0t[p, c*BL + b]
KW_G = KW_X0T + C6 * BL            # 140; g48[p, o*48 + c*NH + h]
KW_ONE = KW_G + O * C6 * NH        # 332: fp16 ones column
KW_LEN = KW_ONE + 4                # 336

# kw32 packing [128, .] fp32: qmask | ones col | boeff (partition 0)
KV_QMASK = 0                       # qmask[p, c*NH + h]
KV_ONES = C6 * NH                  # 48
KV_BOEFF = KV_ONES + 1             # 49 (partition 0 only)
KV_NEG4 = KV_BOEFF + O             # 53: exp bias (-4.0, all partitions)
KV_ID32 = KV_NEG4 + 1              # 54
KV_LEN = KV_ID32 + 128             # 182


def build_program():
    nc = bacc.Bacc(None)

    x_d = nc.declare_dram_parameter("x", [BL, 128, K8, HP], F16, isOutput=False)
    xt_d = nc.declare_dram_parameter("xt", [BL, 128, C6, S], F16, isOutput=False)
    wq_d = nc.declare_dram_parameter("wq", [128, C6, H], F16, isOutput=False)
    wkt_d = nc.declare_dram_parameter("wkt", [128, C6, H], F16, isOutput=False)
    kw16_d = nc.declare_dram_parameter("kw16", [128, KW_LEN], F16, isOutput=False)
    kw32_d = nc.declare_dram_parameter("kw32", [128, KV_LEN], F32, isOutput=False)
    am_d = nc.declare_dram_parameter("am", [128, BL * K8], F16, isOutput=False)
    bq_d = nc.declare_dram_parameter("bq2", [BL, H], F32, isOutput=False)
    out_d = nc.declare_dram_parameter("out", [BL, O], F32, isOutput=True)

    with tile.TileContext(nc) as tc:
        with (
            tc.tile_pool(name="konst", bufs=1) as kp,
            tc.tile_pool(name="work", bufs=1) as wp,
            tc.tile_pool(name="tps", bufs=2, space="PSUM") as tpsp,
            tc.tile_pool(name="acc", bufs=2, space="PSUM") as accp,
            tc.tile_pool(name="jnk", bufs=1, space="PSUM") as jp,
            tc.tile_pool(name="oup", bufs=1, space="PSUM") as op_,
        ):
            # ---- persistent SBUF tiles ----
            kw16 = kp.tile([128, KW_LEN], F16)
            kw32 = kp.tile([128, KV_LEN], F32)
            am16 = kp.tile([128, BL * K8], F16)
            bq32 = kp.tile([BL, H], F32)
            wq_sb = kp.tile([128, C6, H], F16)
            wkt_sb = kp.tile([128, C6, H], F16)
            x_sb = kp.tile([128, BL, K8, HP], F16)
            xt_sb = kp.tile([128, BL, C6, S], F16)

            ident = kw16[:, KW_IDENT : KW_IDENT + 128]
            x0t_v = kw16[:, KW_X0T : KW_G].rearrange("p (c b) -> p c b", c=C6)
            g48_v = kw16[:, KW_G : KW_ONE].rearrange("p (o a) -> p o a", o=O)
            qmask_v = kw32[:, KV_QMASK : KV_ONES].rearrange("p (c h) -> p c h", c=C6)
            ones_v = kw32[:, KV_ONES : KV_ONES + 1]
            one16_v = kw16[:, KW_ONE : KW_ONE + 1]
            boeff_v = kw32[0:1, KV_BOEFF : KV_BOEFF + O]
            id32 = kw32[:, KV_ID32 : KV_ID32 + 128]
            neg4_v = kw32[:, KV_NEG4 : KV_NEG4 + 1]

            # ---- DMA queue (HWDGE; priority-chained, 2 in flight) ----
            dmas = []
            dmas.append(nc.sync.dma_start(out=kw16[:, :], in_=kw16_d[:, :]))
            dmas.append(nc.sync.dma_start(out=kw32[:, :], in_=kw32_d[:, :]))
            dmas.append(nc.sync.dma_start(out=am16[:, :], in_=am_d[:, :]))
            dmas.append(nc.sync.dma_start(out=bq32[:, :], in_=bq_d[:, :]))
            d_wq = []
            d_wkt = []
            for half, (c0, cn) in enumerate(((0, 3), (3, 6))):
                d_wq.append(
                    nc.sync.dma_start(
                        out=wq_sb[:, c0:cn, :],
                        in_=wq_d[:, c0:cn, :],
                    )
                )
            for half, (c0, cn) in enumerate(((0, 3), (3, 6))):
                d_wkt.append(
                    nc.sync.dma_start(
                        out=wkt_sb[:, c0:cn, :],
                        in_=wkt_d[:, c0:cn, :],
                    )
                )
            dmas.extend(d_wq)
            dmas.extend(d_wkt)

            d_xt = {}   # (b, half) -> dma
            d_x = {}    # (b, piece) -> dma; pieces in s-chunks of 128

            def load_xt(b, nh2):
                d_xt[(b, nh2)] = nc.sync.dma_start(
                    out=xt_sb[:, b, :, 512 * nh2 : 512 * (nh2 + 1)],
                    in_=xt_d[b].rearrange("(c p) s -> p c s", p=128)[
                        :, :, 512 * nh2 : 512 * (nh2 + 1)
                    ],
                )

            def load_x(b, k0, kn):
                d_x[(b, k0)] = nc.sync.dma_start(
                    out=x_sb[:, b, k0:kn, :],
                    in_=x_d[b, :, k0:kn, :],
                )

            def load_xt_full(b):
                d_xt[b] = nc.sync.dma_start(
                    out=xt_sb[:, b, :, :],
                    in_=xt_d[b, :, :, :],
                )

            load_xt_full(0)
            load_xt_full(1)
            load_x(0, 0, 4)
            load_x(0, 4, 8)
            load_x(1, 0, 4)
            load_x(1, 4, 6)
            load_x(1, 6, 8)
            dmas.extend(
                [
                    d_xt[0], d_xt[1], d_x[(0, 0)], d_x[(0, 4)],
                    d_x[(1, 0)], d_x[(1, 4)], d_x[(1, 6)],
                ]
            )
            # keep four transfers in flight: hides the ~2.2us per-transfer
            # HWDGE gen + completion-receipt latency while the single ring's
            # FIFO drain keeps bytes landing in priority order
            for i in range(4, len(dmas)):
                _add_dep_helper(
                    dmas[i].ins, dmas[i - 4].ins, sync=True, reason="dma order"
                )

            # ---- PE warmup (HAM unthrottle) while weights stream ----
            warm_ps = jp.tile([128, KW_LEN], F32)
            for _ in range(12):
                nc.tensor.matmul(warm_ps[:, :], ident, kw16[:, :])

            # ---- q~ = X[:,0,:] @ Wq + bq : [BL, H] ----
            q_ps = accp.tile([BL, H], F32, tag="acc", name="q_ps")
            for c in range(C6):
                for n0, nw in ((0, 512), (512, 256)):
                    nc.tensor.matmul(
                        q_ps[:, n0 : n0 + nw],
                        x0t_v[:, c, :],
                        wq_sb[:, c, n0 : n0 + nw],
                        start=(c == 0),
                        stop=(c == C6 - 1),
                    )
            q_sb = wp.tile([BL, H], F32)
            nc.vector.tensor_add(q_sb[:, :], q_ps[:, :], bq32[:, :])

            # ---- qT via PE transposes, fused into Qblk = qT * headmask ----
            qt_ps = tpsp.tile([128, 512], F32, tag="tps", name="qt_ps")
            for c in range(C6):
                nc.tensor.transpose(
                    qt_ps[:, BL * c : BL * (c + 1)],
                    q_sb[:, 128 * c : 128 * (c + 1)],
                    id32[:BL, :BL],
                )
            qblk = wp.tile([128, C6, BL, NH], F16)
            nc.vector.tensor_mul(
                qblk[:, :, :, :],
                qt_ps[:, : C6 * BL]
                .rearrange("p (c b) -> p c b", c=C6)
                .unsqueeze(3)
                .to_broadcast([128, C6, BL, NH]),
                qmask_v.unsqueeze(2).to_broadcast([128, C6, BL, NH]),
            )

            # ---- Z^T [16, 768] = Qblk.T @ WkT, then Z [768, 16] ----
            zt_ps = accp.tile([BL * NH, H], F32, tag="acc", name="zt_ps")
            for jc in range(C6):
                for n0, nw in ((0, 512), (512, 256)):
                    nc.tensor.matmul(
                        zt_ps[:, n0 : n0 + nw],
                        qblk[:, jc, :, :].rearrange("p b h -> p (b h)"),
                        wkt_sb[:, jc, n0 : n0 + nw],
                        start=(jc == 0),
                        stop=(jc == C6 - 1),
                    )
            zt_sb = wp.tile([BL * NH, H], F32)
            nc.vector.tensor_copy(zt_sb[:, :], zt_ps[:, :])
            z_tps = tpsp.tile([128, 512], F32, tag="tps", name="z_tps")
            for c in range(C6):
                nc.tensor.transpose(
                    z_tps[:, 16 * c : 16 * (c + 1)],
                    zt_sb[:, 128 * c : 128 * (c + 1)],
                    id32[: BL * NH, : BL * NH],
                )
            z_sb = wp.tile([128, C6, BL * NH], F16)
            nc.vector.tensor_copy(z_sb[:, :, :], z_tps[:, : C6 * BL * NH].rearrange("p (c a) -> p c a", c=C6))

            # ---- per-batch attention pipeline ----
            probs = wp.tile([NH, BL, S], F32)
            pt_sb = wp.tile([128, BL, K8, NH], F16)

            def scores_half(b, sc_ps, nh2):
                for ic in range(C6):
                    nc.tensor.matmul(
                        sc_ps[:, 512 * nh2 : 512 * (nh2 + 1)],
                        z_sb[:, ic, NH * b : NH * (b + 1)],
                        xt_sb[:, b, ic, 512 * nh2 : 512 * (nh2 + 1)],
                        start=(ic == 0),
                        stop=(ic == C6 - 1),
                    )

            def exp_half(b, sc_ps, nh2):
                # shift-invariant exp; -4 guards fp16 range (|score| <~ 7)
                nc.scalar.activation(
                    probs[:, b, 512 * nh2 : 512 * (nh2 + 1)],
                    sc_ps[:, 512 * nh2 : 512 * (nh2 + 1)],
                    mybir.ActivationFunctionType.Exp,
                    bias=neg4_v[:NH, :],
                    scale=1.0,
                )

            def pt_block(b):
                pt_ps = tpsp.tile([128, 512], F32, tag="tps", name=f"pt_ps{b}")
                for k in range(K8):
                    nc.tensor.transpose(
                        pt_ps[:, NH * k : NH * (k + 1)],
                        probs[:, b, 128 * k : 128 * (k + 1)],
                        id32[:NH, :NH],
                    )
                # attention mask folded into the PSUM evacuation (exact:
                # exp(score - 10000) == 0 == exp(score) * mask in fp32)
                nc.vector.tensor_mul(
                    pt_sb[:, b, :, :],
                    pt_ps[:, : K8 * NH].rearrange("p (k h) -> p k h", k=K8),
                    am16[:, b * K8 : (b + 1) * K8]
                    .unsqueeze(2)
                    .to_broadcast([128, K8, NH]),
                )

            def r_chunks(b, r_ps, k0, kn):
                for k in range(k0, kn):
                    for n0, nw in ((0, 512), (512, HP - 512)):
                        nc.tensor.matmul(
                            r_ps[:, n0 : n0 + nw],
                            pt_sb[:, b, k, :],
                            x_sb[:, b, k, n0 : n0 + nw],
                            start=(k == 0),
                            stop=(k == K8 - 1),
                        )

            def finish(b, r_ps):
                recip = wp.tile([NH, 1], F32, name=f"recip{b}")
                nc.vector.reciprocal(recip[:, :], r_ps[:, RCOL : RCOL + 1])
                rsc = wp.tile([NH, H], F32, name=f"rsc{b}")
                nc.vector.tensor_scalar_mul(
                    rsc[:, :], r_ps[:, :H], recip[:, :]
                )
                rt_ps = tpsp.tile([128, 512], F32, tag="tps", name=f"rt_ps{b}")
                for c in range(C6):
                    nc.tensor.transpose(
                        rt_ps[:, NH * c : NH * (c + 1)],
                        rsc[:, 128 * c : 128 * (c + 1)],
                        id32[:NH, :NH],
                    )
                rt_sb = wp.tile([128, C6 * NH], F16, name=f"rt_sb{b}")
                nc.vector.tensor_copy(rt_sb[:, :], rt_ps[:, : C6 * NH])
                scrap = wp.tile([128, O, C6 * NH], F16, name=f"scrap{b}")
                partials = wp.tile([128, O], F32, name=f"partials{b}")
                for o in range(O):
                    nc.vector.tensor_mul(
                        scrap[:, o, :], rt_sb[:, :], g48_v[:, o, :]
                    )
                    nc.vector.tensor_reduce(
                        partials[:, o : o + 1],
                        scrap[:, o, :],
                        mybir.AxisListType.X,
                        mybir.AluOpType.add,
                    )
                partials16 = wp.tile([128, O], F16, name=f"partials16{b}")
                nc.vector.tensor_copy(partials16[:, :], partials[:, :])
                osum_ps = op_.tile([1, O], F32, tag="out", name=f"osum{b}")
                nc.tensor.matmul(
                    osum_ps[:, :], one16_v, partials16[:, :], start=True, stop=True
                )
                out1 = wp.tile([1, O], F32, name=f"out1{b}")
                nc.vector.tensor_add(out1[:, :], osum_ps[:, :], boeff_v)
                out_sb = wp.tile([1, O], F32, name=f"out_sb{b}")
                nc.vector.tensor_scalar_max(out_sb[:, :], out1[:, :], 0.0)
                nc.scalar.dma_start(out=out_d[b : b + 1, :], in_=out_sb[:, :])

            # interleaved two-batch pipeline: PE chases the DMA queue
            sc_ps0 = accp.tile([NH, S], F32, tag="acc", name="sc_ps0")
            scores_half(0, sc_ps0, 0)
            scores_half(0, sc_ps0, 1)
            exp_half(0, sc_ps0, 0)
            exp_half(0, sc_ps0, 1)
            pt_block(0)
            sc_ps1 = accp.tile([NH, S], F32, tag="acc", name="sc_ps1")
            scores_half(1, sc_ps1, 0)
            scores_half(1, sc_ps1, 1)
            exp_half(1, sc_ps1, 0)
            exp_half(1, sc_ps1, 1)
            r_ps0 = accp.tile([NH, HP], F32, tag="acc", name="r_ps0")
            r_chunks(0, r_ps0, 0, 4)
            pt_block(1)
            r_chunks(0, r_ps0, 4, 8)
            finish(0, r_ps0)
            r_ps1 = accp.tile([NH, HP], F32, tag="acc", name="r_ps1")
            r_chunks(1, r_ps1, 0, 4)
            r_chunks(1, r_ps1, 4, 6)
            r_chunks(1, r_ps1, 6, 8)
            finish(1, r_ps1)

    nc.finalize()
    return nc


_NC_CACHE = None


def _get_program():
    global _NC_CACHE
    if _NC_CACHE is None:
        _NC_CACHE = build_program()
    return _NC_CACHE


def _host_prep(inputs):
    """Weight fusion + fp16 cast + layout prep (host side)."""
    hs = np.asarray(inputs["hidden_states"], np.float32)
    am = np.asarray(inputs["attention_mask"], np.float32)
    Wq = np.asarray(inputs["Wq"], np.float32)
    bq = np.asarray(inputs["bq"], np.float32)
    Wk = np.asarray(inputs["Wk"], np.float32)
    Wv = np.asarray(inputs["Wv"], np.float32)
    bv = np.asarray(inputs["bv"], np.float32)
    Wo = np.asarray(inputs["Wo"], np.float32)
    bo = np.asarray(inputs["bo"], np.float32)

    wq16 = np.ascontiguousarray(
        Wq.astype(NP16).reshape(C6, 128, H).transpose(1, 0, 2)
    )
    wkt16 = np.ascontiguousarray(
        Wk.T.astype(NP16).reshape(C6, 128, H).transpose(1, 0, 2)
    )

    # g48[p, o, c*8+h] = G_h[128c+p, o],  G_h = Wv[:, h] @ Wo[h, :]
    g48 = np.zeros((128, O, C6 * NH), NP16)
    for h in range(NH):
        Gh = (Wv[:, DH * h : DH * (h + 1)] @ Wo[DH * h : DH * (h + 1), :]).astype(
            NP16
        )
        for c in range(C6):
            g48[:, :, c * NH + h] = Gh[128 * c : 128 * (c + 1), :]

    boeff = (bo + bv @ Wo).astype(np.float32)

    # qmask[p, c*8+h]: 1/sqrt(96) where hidden index 128c+p is in head h
    j = np.arange(H)
    qm = np.zeros((H, NH), np.float32)
    qm[j, j // DH] = 1.0 / np.sqrt(np.float32(DH))
    qm = qm.reshape(C6, 128, NH).transpose(1, 0, 2).reshape(128, C6 * NH)

    kw32 = np.zeros((128, KV_LEN), np.float32)
    kw32[:, KV_QMASK:KV_ONES] = qm
    kw32[:, KV_ONES] = 1.0
    kw32[0, KV_BOEFF : KV_BOEFF + O] = boeff
    kw32[:, KV_NEG4] = -4.0
    kw32[:, KV_ID32 : KV_ID32 + 128] = np.eye(128, dtype=np.float32)

    kw16_base = np.zeros((128, KW_LEN), NP16)
    kw16_base[:, KW_IDENT : KW_IDENT + 128] = np.eye(128, dtype=NP16)
    kw16_base[:, KW_G:KW_ONE] = g48.reshape(128, O * C6 * NH)
    kw16_base[:, KW_ONE] = 1.0

    bq2 = np.broadcast_to(bq, (BL, H)).astype(np.float32).copy()

    hs16 = hs.astype(NP16)

    in_maps = []
    for core in range(NCORES):
        b0 = BL * core
        xpad = np.zeros((BL, S, HP), NP16)
        xpad[:, :, :H] = hs16[b0 : b0 + BL]
        xpad[:, :, RCOL] = 1.0
        xpad = np.ascontiguousarray(
            xpad.reshape(BL, K8, 128, HP).transpose(0, 2, 1, 3)
        )
        xt = np.ascontiguousarray(
            hs16[b0 : b0 + BL]
            .transpose(0, 2, 1)
            .reshape(BL, C6, 128, S)
            .transpose(0, 2, 1, 3)
        )

        kw16 = kw16_base.copy()
        # x0t[p, c*BL+b] = X[b0+b, 0, 128c+p]
        kw16[:, KW_X0T:KW_G] = (
            hs16[b0 : b0 + BL, 0, :]
            .reshape(BL, C6, 128)
            .transpose(2, 1, 0)
            .reshape(128, C6 * BL)
        )

        # am[p, b*K8+k] = mask[b0+b, 128k+p]
        amc = (
            am[b0 : b0 + BL, :]
            .reshape(BL, K8, 128)
            .transpose(2, 0, 1)
            .reshape(128, BL * K8)
            .astype(NP16)
        )

        in_maps.append(
            {
                "x": xpad,
                "xt": xt,
                "wq": wq16,
                "wkt": wkt16,
                "kw16": kw16,
                "kw32": kw32,
                "am": np.ascontiguousarray(amc),
                "bq2": bq2,
            }
        )
    return in_maps


def kernel(**inputs) -> np.ndarray:
    nc = _get_program()
    in_maps = _host_prep(inputs)
    res = run_bass_kernel_spmd(nc, in_maps, core_ids=list(range(NCORES)))
    return np.concatenate([r["out"] for r in res.results], axis=0).astype(np.float32)


if __name__ == "__main__":
    rng = np.random.default_rng(0)
    demo = {
        "hidden_states": rng.standard_normal((B, S, H), dtype=np.float32),
        "attention_mask": np.ones((B, S), np.float32),
        "Wq": rng.standard_normal((H, H), dtype=np.float32) / np.sqrt(H),
        "bq": np.zeros(H, np.float32),
        "Wk": rng.standard_normal((H, H), dtype=np.float32) / np.sqrt(H),
        "bk": np.zeros(H, np.float32),
        "Wv": rng.standard_normal((H, H), dtype=np.float32) / np.sqrt(H),
        "bv": np.zeros(H, np.float32),
        "Wo": rng.standard_normal((H, O), dtype=np.float32) / np.sqrt(H),
        "bo": np.zeros(O, np.float32),
    }
    out = kernel(**demo)
    print(out.shape, out.dtype)


# revision 19
# speedup vs baseline: 1.6553x; 1.0219x over previous
"""Trainium2 Bass kernel for BERT-style CLS attention head (v2: fp16 dual-layout).

Model (see harness reference):
  q/k/v projections of hidden [B=16, S=1024, H=768], 8 heads x 96,
  softmax attention, but ONLY the CLS token (query position 0) feeds the
  output projection  out = relu(ctx[:, 0] @ Wo + bo)  with Wo [768, 4].

Algebraic structure exploited on-device (per batch b):
  q~      = X[0] @ Wq + bq                  (only row 0 of Q needed)
  Qblk    [768, 16] = diag-blocked q~/sqrt(96)
  Z^T     [16, 768] = Qblk.T @ Wk^T         (K-projection collapses to a
                                             rank-16 op; bk cancels in
                                             softmax)
  scores  [8, 1024] = Z.T @ X^T             (X^T comes pre-transposed
                                             from DRAM - no on-chip
                                             transposes of X at all)
  probs   = exp(scores - 4)                 (unnormalized; shift cancels)
  pt      = probs^T * mask                  (mask folded into the PSUM
                                             evacuation multiply)
  r       [8, 769]  = pt.T @ [X | 1]       (ones-column of X makes
                                             r[:,768] = rowsum -> softmax
                                             denominator for free)
  out     [4]       = relu(sum_hc rt*g/rho + boeff)  (DVE mult-reduce +
                                             one fp32 matmul; G_h =
                                             Wv_h @ Wo_h host-fused)

All streamed tensors are fp16 (host-side cast + layout only; fp32
accumulation in PSUM). Per-core HBM traffic ~8.6 MB -> ~24 us DMA bound.
Sharding: data-parallel over batch, 2 batches per core on 8 cores.
"""

import numpy as np

from concourse import bacc
import concourse.mybir as mybir
import concourse.tile as tile
from concourse.bass import _add_dep_helper
from concourse.bass_utils import run_bass_kernel_spmd

F32 = mybir.dt.float32
import ml_dtypes
NP16 = ml_dtypes.bfloat16
F16 = mybir.dt.bfloat16
F32R = mybir.dt.float32r


def _r(ap):
    return ap.bitcast(F32R)

B, S, H = 16, 1024, 768
NH, DH, O = 8, 96, 4
NCORES = 8
BL = B // NCORES          # 2 batches per core
C6 = H // 128             # 6 hidden chunks of 128
K8 = S // 128             # 8 sequence chunks of 128
HP = 772                  # padded hidden: col 768 = 1.0 (rowsum), 769.. = 0
RCOL = H                  # index of the ones column in padded X

# kw16 packing [128, .] fp16: ident | x0t | g48
KW_IDENT = 0
KW_X0T = 128                       # x0t[p, c*BL + b]
KW_G = KW_X0T + C6 * BL            # 140; g48[p, o*48 + c*NH + h]
KW_ONE = KW_G + O * C6 * NH        # 332: fp16 ones column
KW_LEN = KW_ONE + 4                # 336

# kw32 packing [128, .] fp32: qmask | ones col | boeff (partition 0)
KV_QMASK = 0                       # qmask[p, c*NH + h]
KV_ONES = C6 * NH                  # 48
KV_BOEFF = KV_ONES + 1             # 49 (partition 0 only)
KV_NEG4 = KV_BOEFF + O             # 53: exp bias (-4.0, all partitions)
KV_ID32 = KV_NEG4 + 1              # 54
KV_LEN = KV_ID32 + 128             # 182


def build_program():
    nc = bacc.Bacc(None)

    x_d = nc.declare_dram_parameter("x", [BL, 128, K8, HP], F16, isOutput=False)
    xt_d = nc.declare_dram_parameter("xt", [BL, 128, C6, S], F16, isOutput=False)
    wq_d = nc.declare_dram_parameter("wq", [128, C6, H], F16, isOutput=False)
    wkt_d = nc.declare_dram_parameter("wkt", [128, C6, H], F16, isOutput=False)
    kw16_d = nc.declare_dram_parameter("kw16", [128, KW_LEN], F16, isOutput=False)
    kw32_d = nc.declare_dram_parameter("kw32", [128, KV_LEN], F32, isOutput=False)
    am_d = nc.declare_dram_parameter("am", [128, BL * K8], F16, isOutput=False)
    bq_d = nc.declare_dram_parameter("bq2", [BL, H], F32, isOutput=False)
    out_d = nc.declare_dram_parameter("out", [BL, O], F32, isOutput=True)

    with tile.TileContext(nc) as tc:
        with (
            tc.tile_pool(name="konst", bufs=1) as kp,
            tc.tile_pool(name="work", bufs=1) as wp,
            tc.tile_pool(name="tps", bufs=2, space="PSUM") as tpsp,
            tc.tile_pool(name="acc", bufs=2, space="PSUM") as accp,
            tc.tile_pool(name="jnk", bufs=1, space="PSUM") as jp,
            tc.tile_pool(name="oup", bufs=1, space="PSUM") as op_,
        ):
            # ---- persistent SBUF tiles ----
            kw16 = kp.tile([128, KW_LEN], F16)
            kw32 = kp.tile([128, KV_LEN], F32)
            am16 = kp.tile([128, BL * K8], F16)
            bq32 = kp.tile([BL, H], F32)
            wq_sb = kp.tile([128, C6, H], F16)
            wkt_sb = kp.tile([128, C6, H], F16)
            x_sb = kp.tile([128, BL, K8, HP], F16)
            xt_sb = kp.tile([128, BL, C6, S], F16)

            ident = kw16[:, KW_IDENT : KW_IDENT + 128]
            x0t_v = kw16[:, KW_X0T : KW_G].rearrange("p (c b) -> p c b", c=C6)
            g48_v = kw16[:, KW_G : KW_ONE].rearrange("p (o a) -> p o a", o=O)
            qmask_v = kw32[:, KV_QMASK : KV_ONES].rearrange("p (c h) -> p c h", c=C6)
            ones_v = kw32[:, KV_ONES : KV_ONES + 1]
            one16_v = kw16[:, KW_ONE : KW_ONE + 1]
            boeff_v = kw32[0:1, KV_BOEFF : KV_BOEFF + O]
            id32 = kw32[:, KV_ID32 : KV_ID32 + 128]
            neg4_v = kw32[:, KV_NEG4 : KV_NEG4 + 1]

            # ---- DMA queue (HWDGE; priority-chained, 2 in flight) ----
            dmas = []
            dmas.append(nc.sync.dma_start(out=kw16[:, :], in_=kw16_d[:, :]))
            dmas.append(nc.sync.dma_start(out=kw32[:, :], in_=kw32_d[:, :]))
            dmas.append(nc.sync.dma_start(out=am16[:, :], in_=am_d[:, :]))
            dmas.append(nc.sync.dma_start(out=bq32[:, :], in_=bq_d[:, :]))
            d_wq = []
            d_wkt = []
            for half, (c0, cn) in enumerate(((0, 3), (3, 6))):
                d_wq.append(
                    nc.sync.dma_start(
                        out=wq_sb[:, c0:cn, :],
                        in_=wq_d[:, c0:cn, :],
                    )
                )
            for half, (c0, cn) in enumerate(((0, 3), (3, 6))):
                d_wkt.append(
                    nc.sync.dma_start(
                        out=wkt_sb[:, c0:cn, :],
                        in_=wkt_d[:, c0:cn, :],
                    )
                )
            dmas.extend(d_wq)
            dmas.extend(d_wkt)

            d_xt = {}   # (b, half) -> dma
            d_x = {}    # (b, piece) -> dma; pieces in s-chunks of 128

            def load_xt(b, nh2):
                d_xt[(b, nh2)] = nc.sync.dma_start(
                    out=xt_sb[:, b, :, 512 * nh2 : 512 * (nh2 + 1)],
                    in_=xt_d[b].rearrange("(c p) s -> p c s", p=128)[
                        :, :, 512 * nh2 : 512 * (nh2 + 1)
                    ],
                )

            def load_x(b, k0, kn):
                d_x[(b, k0)] = nc.sync.dma_start(
                    out=x_sb[:, b, k0:kn, :],
                    in_=x_d[b, :, k0:kn, :],
                )

            def load_xt_full(b):
                d_xt[b] = nc.sync.dma_start(
                    out=xt_sb[:, b, :, :],
                    in_=xt_d[b, :, :, :],
                )

            load_xt_full(0)
            load_xt_full(1)
            load_x(0, 0, 4)
            load_x(0, 4, 8)
            load_x(1, 0, 4)
            load_x(1, 4, 6)
            load_x(1, 6, 8)
            dmas.extend(
                [
                    d_xt[0], d_xt[1], d_x[(0, 0)], d_x[(0, 4)],
                    d_x[(1, 0)], d_x[(1, 4)], d_x[(1, 6)],
                ]
            )
            # keep four transfers in flight: hides the ~2.2us per-transfer
            # HWDGE gen + completion-receipt latency while the single ring's
            # FIFO drain keeps bytes landing in priority order
            for i in range(4, len(dmas)):
                _add_dep_helper(
                    dmas[i].ins, dmas[i - 4].ins, sync=True, reason="dma order"
                )

            # ---- PE warmup (HAM unthrottle) while weights stream ----
            warm_ps = jp.tile([128, KW_LEN], F32)
            for _ in range(12):
                nc.tensor.matmul(warm_ps[:, :], ident, kw16[:, :])

            # ---- q~ = X[:,0,:] @ Wq + bq : [BL, H] ----
            q_ps = accp.tile([BL, H], F32, tag="acc", name="q_ps")
            for c in range(C6):
                for n0, nw in ((0, 512), (512, 256)):
                    nc.tensor.matmul(
                        q_ps[:, n0 : n0 + nw],
                        x0t_v[:, c, :],
                        wq_sb[:, c, n0 : n0 + nw],
                        start=(c == 0),
                        stop=(c == C6 - 1),
                    )
            q_sb = wp.tile([BL, H], F32)
            nc.vector.tensor_add(q_sb[:, :], q_ps[:, :], bq32[:, :])

            # ---- qT via PE transposes, fused into Qblk = qT * headmask ----
            qt_ps = tpsp.tile([128, 512], F32, tag="tps", name="qt_ps")
            for c in range(C6):
                nc.tensor.transpose(
                    qt_ps[:, BL * c : BL * (c + 1)],
                    q_sb[:, 128 * c : 128 * (c + 1)],
                    id32[:BL, :BL],
                )
            qblk = wp.tile([128, C6, BL, NH], F16)
            nc.vector.tensor_mul(
                qblk[:, :, :, :],
                qt_ps[:, : C6 * BL]
                .rearrange("p (c b) -> p c b", c=C6)
                .unsqueeze(3)
                .to_broadcast([128, C6, BL, NH]),
                qmask_v.unsqueeze(2).to_broadcast([128, C6, BL, NH]),
            )

            # ---- Z^T [16, 768] = Qblk.T @ WkT, then Z [768, 16] ----
            zt_ps = accp.tile([BL * NH, H], F32, tag="acc", name="zt_ps")
            for jc in range(C6):
                for n0, nw in ((0, 512), (512, 256)):
                    nc.tensor.matmul(
                        zt_ps[:, n0 : n0 + nw],
                        qblk[:, jc, :, :].rearrange("p b h -> p (b h)"),
                        wkt_sb[:, jc, n0 : n0 + nw],
                        start=(jc == 0),
                        stop=(jc == C6 - 1),
                    )
            zt_sb = wp.tile([BL * NH, H], F32)
            nc.vector.tensor_copy(zt_sb[:, :], zt_ps[:, :])
            z_tps = tpsp.tile([128, 512], F32, tag="tps", name="z_tps")
            for c in range(C6):
                nc.tensor.transpose(
                    z_tps[:, 16 * c : 16 * (c + 1)],
                    zt_sb[:, 128 * c : 128 * (c + 1)],
                    id32[: BL * NH, : BL * NH],
                )
            z_sb = wp.tile([128, C6, BL * NH], F16)
            nc.vector.tensor_copy(z_sb[:, :, :], z_tps[:, : C6 * BL * NH].rearrange("p (c a) -> p c a", c=C6))

            # ---- per-batch attention pipeline ----
            probs = wp.tile([NH, BL, S], F32)
            pt_sb = wp.tile([128, BL, K8, NH], F16)

            def scores_half(b, sc_ps, nh2):
                for ic in range(C6):
                    nc.tensor.matmul(
                        sc_ps[:, 512 * nh2 : 512 * (nh2 + 1)],
                        z_sb[:, ic, NH * b : NH * (b + 1)],
                        xt_sb[:, b, ic, 512 * nh2 : 512 * (nh2 + 1)],
                        start=(ic == 0),
                        stop=(ic == C6 - 1),
                    )

            def exp_half(b, sc_ps, nh2):
                # shift-invariant exp; -4 guards fp16 range (|score| <~ 7)
                nc.scalar.activation(
                    probs[:, b, 512 * nh2 : 512 * (nh2 + 1)],
                    sc_ps[:, 512 * nh2 : 512 * (nh2 + 1)],
                    mybir.ActivationFunctionType.Exp,
                    bias=neg4_v[:NH, :],
                    scale=1.0,
                )

            def pt_block(b):
                pt_ps = tpsp.tile([128, 512], F32, tag="tps", name=f"pt_ps{b}")
                for k in range(K8):
                    nc.tensor.transpose(
                        pt_ps[:, NH * k : NH * (k + 1)],
                        probs[:, b, 128 * k : 128 * (k + 1)],
                        id32[:NH, :NH],
                    )
                # attention mask folded into the PSUM evacuation (exact:
                # exp(score - 10000) == 0 == exp(score) * mask in fp32)
                nc.vector.tensor_mul(
                    pt_sb[:, b, :, :],
                    pt_ps[:, : K8 * NH].rearrange("p (k h) -> p k h", k=K8),
                    am16[:, b * K8 : (b + 1) * K8]
                    .unsqueeze(2)
                    .to_broadcast([128, K8, NH]),
                )

            def r_chunks(b, r_ps, k0, kn):
                for k in range(k0, kn):
                    for n0, nw in ((0, 512), (512, HP - 512)):
                        nc.tensor.matmul(
                            r_ps[:, n0 : n0 + nw],
                            pt_sb[:, b, k, :],
                            x_sb[:, b, k, n0 : n0 + nw],
                            start=(k == 0),
                            stop=(k == K8 - 1),
                        )

            def finish(b, r_ps):
                recip = wp.tile([NH, 1], F32, name=f"recip{b}")
                nc.vector.reciprocal(recip[:, :], r_ps[:, RCOL : RCOL + 1])
                rsc = wp.tile([NH, H], F32, name=f"rsc{b}")
                nc.vector.tensor_scalar_mul(
                    rsc[:, 0:384], r_ps[:, 0:384], recip[:, :]
                )
                nc.scalar.activation(
                    rsc[:, 384:H],
                    r_ps[:, 384:H],
                    mybir.ActivationFunctionType.Copy,
                    bias=0.0,
                    scale=recip[:, :],
                )
                rt_ps = tpsp.tile([128, 512], F32, tag="tps", name=f"rt_ps{b}")
                for c in range(C6):
                    nc.tensor.transpose(
                        rt_ps[:, NH * c : NH * (c + 1)],
                        rsc[:, 128 * c : 128 * (c + 1)],
                        id32[:NH, :NH],
                    )
                rt_sb = wp.tile([128, C6 * NH], F16, name=f"rt_sb{b}")
                nc.vector.tensor_copy(rt_sb[:, :], rt_ps[:, : C6 * NH])
                scrap = wp.tile([128, O, C6 * NH], F16, name=f"scrap{b}")
                partials = wp.tile([128, O], F32, name=f"partials{b}")
                nc.vector.tensor_mul(
                    scrap[:, :, :],
                    rt_sb[:, :].unsqueeze(1).to_broadcast([128, O, C6 * NH]),
                    g48_v[:, :, :],
                )
                nc.vector.tensor_reduce(
                    partials[:, :].unsqueeze(2),
                    scrap[:, :, :],
                    mybir.AxisListType.X,
                    mybir.AluOpType.add,
                )
                partials16 = wp.tile([128, O], F16, name=f"partials16{b}")
                nc.vector.tensor_copy(partials16[:, :], partials[:, :])
                osum_ps = op_.tile([1, O], F32, tag="out", name=f"osum{b}")
                nc.tensor.matmul(
                    osum_ps[:, :], one16_v, partials16[:, :], start=True, stop=True
                )
                out1 = wp.tile([1, O], F32, name=f"out1{b}")
                nc.vector.tensor_add(out1[:, :], osum_ps[:, :], boeff_v)
                out_sb = wp.tile([1, O], F32, name=f"out_sb{b}")
                nc.vector.tensor_scalar_max(out_sb[:, :], out1[:, :], 0.0)
                nc.scalar.dma_start(out=out_d[b : b + 1, :], in_=out_sb[:, :])

            # interleaved two-batch pipeline: PE chases the DMA queue
            sc_ps0 = accp.tile([NH, S], F32, tag="acc", name="sc_ps0")
            scores_half(0, sc_ps0, 0)
            scores_half(0, sc_ps0, 1)
            exp_half(0, sc_ps0, 0)
            exp_half(0, sc_ps0, 1)
            pt_block(0)
            for _ in range(5):
                nc.tensor.matmul(warm_ps[:, :], ident, kw16[:, :])
            sc_ps1 = accp.tile([NH, S], F32, tag="acc", name="sc_ps1")
            scores_half(1, sc_ps1, 0)
            scores_half(1, sc_ps1, 1)
            exp_half(1, sc_ps1, 0)
            exp_half(1, sc_ps1, 1)
            r_ps0 = accp.tile([NH, HP], F32, tag="acc", name="r_ps0")
            r_chunks(0, r_ps0, 0, 4)
            pt_block(1)
            r_chunks(0, r_ps0, 4, 8)
            r_ps1 = accp.tile([NH, HP], F32, tag="acc", name="r_ps1")
            r_chunks(1, r_ps1, 0, 4)
            r_chunks(1, r_ps1, 4, 6)
            r_chunks(1, r_ps1, 6, 8)
            finish(0, r_ps0)
            finish(1, r_ps1)

    nc.finalize()
    return nc


_NC_CACHE = None


def _get_program():
    global _NC_CACHE
    if _NC_CACHE is None:
        _NC_CACHE = build_program()
    return _NC_CACHE


def _host_prep(inputs):
    """Weight fusion + fp16 cast + layout prep (host side)."""
    hs = np.asarray(inputs["hidden_states"], np.float32)
    am = np.asarray(inputs["attention_mask"], np.float32)
    Wq = np.asarray(inputs["Wq"], np.float32)
    bq = np.asarray(inputs["bq"], np.float32)
    Wk = np.asarray(inputs["Wk"], np.float32)
    Wv = np.asarray(inputs["Wv"], np.float32)
    bv = np.asarray(inputs["bv"], np.float32)
    Wo = np.asarray(inputs["Wo"], np.float32)
    bo = np.asarray(inputs["bo"], np.float32)

    wq16 = np.ascontiguousarray(
        Wq.astype(NP16).reshape(C6, 128, H).transpose(1, 0, 2)
    )
    wkt16 = np.ascontiguousarray(
        Wk.T.astype(NP16).reshape(C6, 128, H).transpose(1, 0, 2)
    )

    # g48[p, o, c*8+h] = G_h[128c+p, o],  G_h = Wv[:, h] @ Wo[h, :]
    g48 = np.zeros((128, O, C6 * NH), NP16)
    for h in range(NH):
        Gh = (Wv[:, DH * h : DH * (h + 1)] @ Wo[DH * h : DH * (h + 1), :]).astype(
            NP16
        )
        for c in range(C6):
            g48[:, :, c * NH + h] = Gh[128 * c : 128 * (c + 1), :]

    boeff = (bo + bv @ Wo).astype(np.float32)

    # qmask[p, c*8+h]: 1/sqrt(96) where hidden index 128c+p is in head h
    j = np.arange(H)
    qm = np.zeros((H, NH), np.float32)
    qm[j, j // DH] = 1.0 / np.sqrt(np.float32(DH))
    qm = qm.reshape(C6, 128, NH).transpose(1, 0, 2).reshape(128, C6 * NH)

    kw32 = np.zeros((128, KV_LEN), np.float32)
    kw32[:, KV_QMASK:KV_ONES] = qm
    kw32[:, KV_ONES] = 1.0
    kw32[0, KV_BOEFF : KV_BOEFF + O] = boeff
    kw32[:, KV_NEG4] = -4.0
    kw32[:, KV_ID32 : KV_ID32 + 128] = np.eye(128, dtype=np.float32)

    kw16_base = np.zeros((128, KW_LEN), NP16)
    kw16_base[:, KW_IDENT : KW_IDENT + 128] = np.eye(128, dtype=NP16)
    kw16_base[:, KW_G:KW_ONE] = g48.reshape(128, O * C6 * NH)
    kw16_base[:, KW_ONE] = 1.0

    bq2 = np.broadcast_to(bq, (BL, H)).astype(np.float32).copy()

    hs16 = hs.astype(NP16)

    in_maps = []
    for core in range(NCORES):
        b0 = BL * core
        xpad = np.zeros((BL, S, HP), NP16)
        xpad[:, :, :H] = hs16[b0 : b0 + BL]
        xpad[:, :, RCOL] = 1.0
        xpad = np.ascontiguousarray(
            xpad.reshape(BL, K8, 128, HP).transpose(0, 2, 1, 3)
        )
        xt = np.ascontiguousarray(
            hs16[b0 : b0 + BL]
            .transpose(0, 2, 1)
            .reshape(BL, C6, 128, S)
            .transpose(0, 2, 1, 3)
        )

        kw16 = kw16_base.copy()
        # x0t[p, c*BL+b] = X[b0+b, 0, 128c+p]
        kw16[:, KW_X0T:KW_G] = (
            hs16[b0 : b0 + BL, 0, :]
            .reshape(BL, C6, 128)
            .transpose(2, 1, 0)
            .reshape(128, C6 * BL)
        )

        # am[p, b*K8+k] = mask[b0+b, 128k+p]
        amc = (
            am[b0 : b0 + BL, :]
            .reshape(BL, K8, 128)
            .transpose(2, 0, 1)
            .reshape(128, BL * K8)
            .astype(NP16)
        )

        in_maps.append(
            {
                "x": xpad,
                "xt": xt,
                "wq": wq16,
                "wkt": wkt16,
                "kw16": kw16,
                "kw32": kw32,
                "am": np.ascontiguousarray(amc),
                "bq2": bq2,
            }
        )
    return in_maps


def kernel(**inputs) -> np.ndarray:
    nc = _get_program()
    in_maps = _host_prep(inputs)
    res = run_bass_kernel_spmd(nc, in_maps, core_ids=list(range(NCORES)))
    return np.concatenate([r["out"] for r in res.results], axis=0).astype(np.float32)


if __name__ == "__main__":
    rng = np.random.default_rng(0)
    demo = {
        "hidden_states": rng.standard_normal((B, S, H), dtype=np.float32),
        "attention_mask": np.ones((B, S), np.float32),
        "Wq": rng.standard_normal((H, H), dtype=np.float32) / np.sqrt(H),
        "bq": np.zeros(H, np.float32),
        "Wk": rng.standard_normal((H, H), dtype=np.float32) / np.sqrt(H),
        "bk": np.zeros(H, np.float32),
        "Wv": rng.standard_normal((H, H), dtype=np.float32) / np.sqrt(H),
        "bv": np.zeros(H, np.float32),
        "Wo": rng.standard_normal((H, O), dtype=np.float32) / np.sqrt(H),
        "bo": np.zeros(O, np.float32),
    }
    out = kernel(**demo)
    print(out.shape, out.dtype)
